# revision 10
# baseline (speedup 1.0000x reference)
"""Causal self-attention, tensor-parallel over heads across 8 TRN2 NeuronCores.

B=2, T=2048, C=1024, H=16 heads, D=64. Each core owns 2 heads (128 cols of C)
for both batches. qT/kT hold both heads stacked on partitions (h0 rows 0:64,
h1 rows 64:128) so score matmuls pair the two heads on disjoint PE row-groups
with no duplicated bias writes. v is produced transposed (w-stationary, wide
moving) then PE-transposed to natural layout via an identity matmul. One A2A
per batch redistributes unnormalized y^T (+ softmax normalizer row) into row
shards; each core then normalizes and computes a disjoint 512-row slice of the
Wo projection. Sends ride the GpSimd queue, unpacks ride the Sync queue so
collective-completion waits never head-block compute-critical queues.
bf16 matmuls, fp32 PSUM accumulation.
"""

import sys

sys.path.insert(0, "/opt/trn_rl_repo")

import numpy as np
import ml_dtypes

import concourse.bass as bass
import concourse.bacc as bacc
import concourse.mybir as mybir
from concourse.tile import TileContext
from concourse.masks import make_identity
from concourse import bass_utils

BF16 = mybir.dt.bfloat16
F32 = mybir.dt.float32
NPBF16 = ml_dtypes.bfloat16

B, T, C, H, D = 2, 2048, 1024, 16, 64
NCORES = 8
HL = H // NCORES          # heads per core = 2
COLS = HL * D             # 128 head-cols per core
KT = C // 128             # 8 contraction k-tiles
NCH = T // 512            # 4 query chunks of 512 per batch
NT = T // 128             # 16 key tiles of 128 per batch
VW = D + 1                # 65: v columns + ones column
SH = 72                   # per-head subshard rows (65 used, pad to 4KB rows)
SH2 = 2 * SH              # both heads stacked per destination shard
ROWS_PER_CORE = B * T // NCORES  # 512 output rows per core

MASK_NEG = -60000.0
SCALE = 1.0 / np.sqrt(np.float32(D))

Exp = mybir.ActivationFunctionType.Exp
Ident = mybir.ActivationFunctionType.Identity


def build_nc():
    nc = bacc.Bacc(
        "TRN2",
        target_bir_lowering=False,
        debug=False,
        enable_asserts=False,
        num_devices=NCORES,
    )
    xT = nc.dram_tensor("xT", [C, B * T], BF16, kind="ExternalInput")
    # weights pre-tiled on host to [128, k-tile blocks] for contiguous DMA
    wq = nc.dram_tensor("wq", [128, KT * COLS], BF16, kind="ExternalInput")
    wk = nc.dram_tensor("wk", [128, KT * COLS], BF16, kind="ExternalInput")
    wv = nc.dram_tensor("wv", [128, KT * COLS], BF16, kind="ExternalInput")
    # wo rows pre-permuted h-major [h, core, 64] and pre-tiled likewise
    wo = nc.dram_tensor("wo", [128, KT * C], BF16, kind="ExternalInput")
    bqk = nc.dram_tensor("bqk", [COLS, 2], F32, kind="ExternalInput")
    bv = nc.dram_tensor("bv", [COLS, 1], F32, kind="ExternalInput")
    bo = nc.dram_tensor("bo", [1, C], BF16, kind="ExternalInput")
    mtri = nc.dram_tensor("mtri", [128, 128], BF16, kind="ExternalInput")
    emat = nc.dram_tensor("emat", [NCORES, C], BF16, kind="ExternalInput")
    # one A2A per batch; dst shard = [h0 y^T+r | pad | h1 y^T+r | pad] x 256 tok
    send = [
        nc.dram_tensor(f"a2a_send{b}", [NCORES * SH2, 256], BF16) for b in range(B)
    ]
    recv = [
        nc.dram_tensor(f"a2a_recv{b}", [NCORES * SH2, 256], BF16) for b in range(B)
    ]
    wsend = nc.dram_tensor("warm_send", [NCORES * 2, 512], BF16)
    wrecv = nc.dram_tensor("warm_recv", [NCORES * 2, 512], BF16)
    out = nc.dram_tensor("out", [ROWS_PER_CORE, C], F32, kind="ExternalOutput")

    add = mybir.AluOpType.add
    mult = mybir.AluOpType.mult

    def stage_a_batch(b, x_sb, psA, psT, cs):
        # q/k projections into transposed, head-stacked layout
        for wsb, dsb, bcol in ((cs["wq"], cs["qT"], 0), (cs["wk"], cs["kT"], 1)):
            for n in range(NCH):
                ps = psA.tile([128, 512], F32, tag="psA", name="psA", bufs=4)
                col = b * T + n * 512
                for k in range(KT):
                    nc.tensor.matmul(
                        ps[:],
                        wsb[:, k * COLS : (k + 1) * COLS],
                        x_sb[k][:, col : col + 512],
                        start=(k == 0),
                        stop=(k == KT - 1),
                    )
                nc.scalar.activation(
                    dsb[:, col : col + 512],
                    ps[:],
                    Ident,
                    bias=cs["bqk"][:, bcol : bcol + 1],
                )
        # v^T (w-stationary, wide moving), then PE-transpose to natural layout
        for n in range(NCH):
            ps = psA.tile([128, 512], F32, tag="psA", name="psA", bufs=4)
            col = b * T + n * 512
            for k in range(KT):
                nc.tensor.matmul(
                    ps[:],
                    cs["wv"][:, k * COLS : (k + 1) * COLS],
                    x_sb[k][:, col : col + 512],
                    start=(k == 0),
                    stop=(k == KT - 1),
                )
            nc.scalar.activation(
                cs["vT"][:, col : col + 512], ps[:], Ident, bias=cs["bv"][:, 0:1]
            )
        for m in range(NT):
            pv = psT.tile([128, 128], F32, tag="psT", name="psT", bufs=2)
            col = b * T + m * 128
            # out = vT_tile^T via moving identity
            nc.tensor.matmul(
                pv[:], cs["vT"][:, col : col + 128], cs["ident"][:],
                start=True, stop=True,
            )
            vi = (b * NT + m) * HL
            nc.vector.tensor_copy(
                out=cs["v"][:, vi : vi + HL, 0:D],
                in_=pv[:, :].rearrange("p (h d) -> p h d", h=HL),
            )

    def scores_chunk(b, n, pts, ptp, psS, cs):
        """Generator: score tiles + exp for query chunk (b, n); h0/h1 paired
        on disjoint PE row-groups. Appends pt (=exp(scaled scores)^T) tiles to
        the caller's per-head lists; yields after each key tile."""
        qT, kT = cs["qT"], cs["kT"]
        qcol = b * T + n * 512
        for m in range(4 * n + 4):
            kcol = b * T + m * 128
            j = m - 4 * n  # >= 0 only for the diagonal block tiles
            for h in range(HL):
                ps = psS.tile(
                    [128, 512], F32, tag=f"psS{h}", name=f"psS{h}", bufs=3
                )
                if j < 0:
                    nc.tensor.matmul(
                        ps[:],
                        kT[h * D : (h + 1) * D, kcol : kcol + 128],
                        qT[h * D : (h + 1) * D, qcol : qcol + 512],
                        start=True,
                        stop=True,
                    )
                    pt = ptp.tile([128, 512], BF16, tag="pt", name="pt", bufs=56)
                    nc.scalar.activation(pt[:], ps[:], Exp, scale=float(SCALE))
                else:
                    # diagonal: one matmul for cols j*128.., causal mask of the
                    # j-th 128-block via vector add, zero-fill to the left
                    nc.tensor.matmul(
                        ps[:, j * 128 :],
                        kT[h * D : (h + 1) * D, kcol : kcol + 128],
                        qT[h * D : (h + 1) * D, qcol + j * 128 : qcol + 512],
                        start=True,
                        stop=True,
                    )
                    nc.vector.tensor_tensor(
                        ps[:, j * 128 : (j + 1) * 128],
                        ps[:, j * 128 : (j + 1) * 128],
                        cs["mtri"][:],
                        add,
                    )
                    pt = ptp.tile([128, 512], BF16, tag="pt", name="pt", bufs=56)
                    if j > 0:
                        nc.gpsimd.memset(pt[:, 0 : j * 128], 0.0)
                    nc.scalar.activation(
                        pt[:, j * 128 :], ps[:, j * 128 :], Exp, scale=float(SCALE)
                    )
                pts[h].append(pt)
            yield

    def av_chunk(b, n, pts, psY, nrm, cs):
        """Generator: y^T (+ normalizer row) = v_aug^T @ P^T for chunk (b, n),
        then ship unnormalized y^T plus the r row (receive side divides).
        Yields after each matmul so the driver can interleave score pairs."""
        last = 4 * n + 3
        for h in range(HL):
            py = psY.tile([VW, 512], F32, tag="psY", name="psY", bufs=2)
            for m in range(4 * n + 4):
                vi = (b * NT + m) * HL + h
                nc.tensor.matmul(
                    py[:],
                    cs["v"][:, vi : vi + 1, :],
                    pts[h][m][:],
                    start=(m == 0),
                    stop=(m == last),
                )
                if m < last:
                    yield
            yn = nrm.tile([VW, 512], BF16, tag="yn", name="yn", bufs=6)
            nc.vector.tensor_copy(out=yn[:], in_=py[:])
            for p in range(2):
                dst = 2 * n + p
                o = send[b][dst * SH2 + h * SH : dst * SH2 + h * SH + VW, :]
                nc.gpsimd.dma_start(out=o, in_=yn[:, p * 256 : (p + 1) * 256])
            yield

    def a2a(b):
        nc.gpsimd.collective_compute(
            "AllToAll",
            mybir.AluOpType.bypass,
            replica_groups=[list(range(NCORES))],
            ins=[send[b][:]],
            outs=[recv[b][:]],
        )

    def make_inv(cp, r_sb, tag):
        r_f = cp.tile([NCORES, 256], F32, tag=f"rf{tag}", name=f"rf{tag}")
        nc.vector.tensor_copy(out=r_f[:], in_=r_sb[:])
        invf = cp.tile([NCORES, 256], F32, tag=f"invf{tag}", name=f"invf{tag}")
        nc.vector.reciprocal_approx_fast(out=invf[:], in_=r_f[:])
        inv = cp.tile([NCORES, 256], BF16, tag=f"inv{tag}", name=f"inv{tag}")
        nc.vector.tensor_copy(out=inv[:], in_=invf[:])
        return inv

    def stage_c_batch(b, cp, psC, cs, y_sb, yn_sb):
        """After a2a(b): unpack, normalize, Wo rows 2b,2b+1, store."""
        c0 = b * 256
        rv = recv[b].rearrange("(k p dr) c -> p dr k c", k=4, p=2)
        for p in range(2):
            for hh, k0 in ((0, 0), (SH, 4)):
                nc.sync.dma_start(
                    out=y_sb[p * D : p * D + D, k0 * 512 : (k0 + 4) * 512].rearrange(
                        "d (k c) -> d k c", k=4
                    )[:, :, c0 : c0 + 256],
                    in_=rv[p, hh : hh + D, :, :],
                )
        rr = recv[b].rearrange("(s dr) c -> s dr c", s=NCORES)
        r0_sb = cp.tile([NCORES, 256], BF16, tag=f"rsb0{b}", name=f"rsb0{b}")
        r1_sb = cp.tile([NCORES, 256], BF16, tag=f"rsb1{b}", name=f"rsb1{b}")
        nc.sync.dma_start(
            out=r0_sb[:].rearrange("s (o c) -> s o c", o=1), in_=rr[:, D : D + 1, :]
        )
        nc.sync.dma_start(
            out=r1_sb[:].rearrange("s (o c) -> s o c", o=1),
            in_=rr[:, SH + D : SH + D + 1, :],
        )
        inv0 = make_inv(cp, r0_sb, f"0{b}")
        inv1 = make_inv(cp, r1_sb, f"1{b}")
        for k in range(8):
            inv = inv0 if k < 4 else inv1
            pb = psC.tile([128, 256], F32, tag="psB", name="psB", bufs=2)
            nc.tensor.matmul(
                pb[:],
                cs["emat"][:, (k % 4) * 128 : (k % 4 + 1) * 128],
                inv[:],
                start=True,
                stop=True,
            )
            nc.vector.tensor_tensor(
                yn_sb[:, k * 512 + c0 : k * 512 + c0 + 256],
                y_sb[:, k * 512 + c0 : k * 512 + c0 + 256],
                pb[:],
                mult,
            )
        for r in (2 * b, 2 * b + 1):
            for o in range(C // 512):
                pc = psC.tile([128, 512], F32, tag="psC", name="psC", bufs=4)
                for k in range(KT):
                    nc.tensor.matmul(
                        pc[:],
                        yn_sb[:, k * 512 + r * 128 : k * 512 + r * 128 + 128],
                        cs["wo"][:, k * C + o * 512 : k * C + (o + 1) * 512],
                        start=(k == 0),
                        stop=False,
                    )
                nc.tensor.matmul(
                    pc[:],
                    cs["ones"][0:1, :],
                    cs["bo"][0:1, o * 512 : (o + 1) * 512],
                    start=False,
                    stop=True,
                )
                osb = cp.tile([128, 512], F32, tag="osb", name="osb", bufs=3)
                nc.vector.tensor_copy(out=osb[:], in_=pc[:])
                nc.scalar.dma_start(
                    out=out[r * 128 : (r + 1) * 128, o * 512 : (o + 1) * 512],
                    in_=osb[:],
                )

    with TileContext(nc) as tc:
        with tc.tile_pool(name="persist", bufs=1) as pp:
            cs = {}
            # warmup collective first: absorbs the ~11us first-trigger latency
            nc.gpsimd.collective_compute(
                "AllToAll",
                mybir.AluOpType.bypass,
                replica_groups=[list(range(NCORES))],
                ins=[wsend[:]],
                outs=[wrecv[:]],
            )
            # tiny constants FIRST: a late-landing constant can head-block the
            # PE queue (first scheduled LDWEIGHTS waits on it)
            cs["ones"] = pp.tile([1, 128], BF16, tag="ones", name="ones")
            nc.vector.memset(cs["ones"][:], 1.0)
            cs["bqk"] = pp.tile([COLS, 2], F32, tag="bqk", name="bqk")
            nc.sync.dma_start(out=cs["bqk"][:], in_=bqk[:])
            cs["bv"] = pp.tile([COLS, 1], F32, tag="bv", name="bv")
            nc.sync.dma_start(out=cs["bv"][:], in_=bv[:])
            cs["bo"] = pp.tile([1, C], BF16, tag="bo", name="bo")
            nc.scalar.dma_start(out=cs["bo"][:], in_=bo[:])
            cs["mtri"] = pp.tile([128, 128], BF16, tag="mtri", name="mtri")
            nc.scalar.dma_start(out=cs["mtri"][:], in_=mtri[:])
            cs["emat"] = pp.tile([NCORES, C], BF16, tag="emat", name="emat")
            nc.gpsimd.dma_start(out=cs["emat"][:], in_=emat[:])
            cs["ident"] = pp.tile([128, 128], BF16, tag="ident", name="ident")
            make_identity(nc, cs["ident"][:])

            # weights next (needed with first x tiles)
            cs["wq"] = pp.tile([128, KT * COLS], BF16, tag="wq", name="wq")
            cs["wk"] = pp.tile([128, KT * COLS], BF16, tag="wk", name="wk")
            cs["wv"] = pp.tile([128, KT * COLS], BF16, tag="wv", name="wv")
            nc.sync.dma_start(out=cs["wq"][:], in_=wq[:])
            nc.scalar.dma_start(out=cs["wk"][:], in_=wk[:])
            nc.gpsimd.dma_start(out=cs["wv"][:], in_=wv[:])

            with tc.tile_pool(name="xp", bufs=1) as xp:
                # x k-tiles split into 512-col chunks over 4 engine DMA queues
                # (not tensor: DMA issues there would head-block the first
                # matmuls), first-consumed columns first
                x_sb = [
                    xp.tile([128, B * T], BF16, tag=f"x{k}", name=f"x{k}")
                    for k in range(KT)
                ]
                xq = [nc.sync, nc.scalar, nc.gpsimd]
                qi = 0
                for c in range(B * T // 512):
                    for k in range(KT):
                        xq[qi % 3].dma_start(
                            out=x_sb[k][:, c * 512 : (c + 1) * 512],
                            in_=xT[k * 128 : (k + 1) * 128, c * 512 : (c + 1) * 512],
                        )
                        qi += 1

                cs["qT"] = pp.tile([128, B * T], BF16, tag="qT", name="qT")
                cs["kT"] = pp.tile([128, B * T], BF16, tag="kT", name="kT")
                cs["vT"] = pp.tile([128, B * T], BF16, tag="vT", name="vT")
                cs["v"] = pp.tile([128, B * NT * HL, VW], BF16, tag="v", name="v")
                nc.gpsimd.memset(cs["v"][:], 1.0)  # presets the ones columns

                # wo loaded last (not needed until stage C)
                cs["wo"] = pp.tile([128, KT * C], BF16, tag="wo", name="wo")
                nc.scalar.dma_start(out=cs["wo"][:], in_=wo[:])

                with tc.tile_pool(name="psA", bufs=1, space="PSUM") as psA, \
                     tc.tile_pool(name="psT", bufs=1, space="PSUM") as psT:
                    for b in range(B):
                        stage_a_batch(b, x_sb, psA, psT, cs)

            with tc.tile_pool(name="pt", bufs=1) as ptp, tc.tile_pool(
                name="psS", bufs=1, space="PSUM"
            ) as psS, tc.tile_pool(
                name="psY", bufs=1, space="PSUM"
            ) as psY, tc.tile_pool(
                name="nrm", bufs=1
            ) as nrm:
                # software pipeline: interleave score pairs of chunk u with the
                # AV matmuls of chunk u-1 at instruction level, so the
                # scalar-engine exps (slower than paired score production)
                # overlap the AV matmul stream instead of serializing
                prev_pts = None
                for u in range(B * NCH + 1):
                    sg = None
                    if u < B * NCH:
                        cur_pts = ([], [])
                        sg = scores_chunk(u // NCH, u % NCH, cur_pts, ptp, psS, cs)
                    ag = None
                    if u > 0:
                        pb, pn = (u - 1) // NCH, (u - 1) % NCH
                        ag = av_chunk(pb, pn, prev_pts, psY, nrm, cs)
                    if sg is not None and ag is not None:
                        s_steps = 4 * (u % NCH) + 4
                        a_steps = 2 * (4 * pn + 4)
                        acc = 0.0
                        for _ in range(s_steps):
                            next(sg, None)
                            acc += a_steps / s_steps
                            while acc >= 1.0:
                                next(ag, None)
                                acc -= 1.0
                    if sg is not None:
                        for _ in sg:
                            pass
                    if ag is not None:
                        for _ in ag:
                            pass
                        if pn == NCH - 1:
                            a2a(pb)
                    prev_pts = cur_pts
            with tc.tile_pool(name="cp", bufs=1) as cp, tc.tile_pool(
                name="psC", bufs=1, space="PSUM"
            ) as psC:
                y_sb = cp.tile([128, 8 * 512], BF16, tag="ysb", name="ysb")
                yn_sb = cp.tile([128, 8 * 512], BF16, tag="ynsb", name="ynsb")
                for b in range(B):
                    stage_c_batch(b, cp, psC, cs, y_sb, yn_sb)
    nc.compile()
    return nc


def make_in_maps(x, mask, Wq, bq, Wk, bk, Wv, bv, Wo, bo):
    xT = np.ascontiguousarray(
        x.astype(np.float32).transpose(2, 0, 1).reshape(C, B * T)
    ).astype(NPBF16)
    mtri = np.where(
        np.arange(128)[:, None] > np.arange(128)[None, :], MASK_NEG, 0.0
    ).astype(NPBF16)
    # Wo rows permuted h-major: new row order = [core0 h0 d0..63, core1 h0, ...,
    # core7 h0, core0 h1, ..., core7 h1]
    perm = np.concatenate(
        [
            np.arange(c * COLS + h * D, c * COLS + h * D + D)
            for h in range(HL)
            for c in range(NCORES)
        ]
    )
    def pretile(w):
        # [C, width] -> [128, KT*width] with k-tile blocks along free axis
        width = w.shape[1]
        return np.ascontiguousarray(
            w.reshape(KT, 128, width).transpose(1, 0, 2).reshape(128, KT * width)
        )
    wo_b = pretile(Wo[perm].astype(NPBF16))
    # emat[s, c] = 1 iff shard s supplies the softmax normalizer for y-column
    # position c of the per-head k-chunk layout (chunk k = shards 2k, 2k+1)
    cols = np.arange(C)
    emat_np = (
        np.arange(NCORES)[:, None] == (2 * (cols // 128) + (cols % 128) // 64)
    ).astype(NPBF16)
    bo_b = bo.reshape(1, C).astype(NPBF16)
    in_maps = []
    for c in range(NCORES):
        cslice = slice(c * COLS, (c + 1) * COLS)
        in_maps.append(
            {
                "xT": xT,
                "wq": pretile(Wq[:, cslice].astype(NPBF16)),
                "wk": pretile(Wk[:, cslice].astype(NPBF16)),
                "wv": pretile(Wv[:, cslice].astype(NPBF16)),
                "wo": wo_b,
                "bqk": np.stack([bq[cslice], bk[cslice]], axis=1).astype(
                    np.float32
                ),
                "bv": bv[cslice].reshape(COLS, 1).astype(np.float32),
                "bo": bo_b,
                "mtri": mtri,
                "emat": emat_np,
            }
        )
    return in_maps


_CACHED_NC = None


def run(inputs, trace=False, **kw):
    global _CACHED_NC
    if _CACHED_NC is None:
        _CACHED_NC = build_nc()
    in_maps = make_in_maps(**inputs)
    res = bass_utils.run_bass_kernel_spmd(
        _CACHED_NC, in_maps, core_ids=list(range(NCORES)), trace=trace, **kw
    )
    outs = [np.asarray(res.results[c]["out"]) for c in range(NCORES)]
    full = np.empty((B, T, C), np.float32)
    for j in range(NCORES):
        full[0, 256 * j : 256 * (j + 1)] = outs[j][0:256]
        full[1, 256 * j : 256 * (j + 1)] = outs[j][256:512]
    return full, res


def kernel(**inputs):
    full, _ = run(inputs, trace=False)
    return full


# revision 11
# speedup vs baseline: 1.2398x; 1.2398x over previous
"""Causal self-attention, tensor-parallel over heads across 8 TRN2 NeuronCores.

B=2, T=2048, C=1024, H=16 heads, D=64. Each core owns 2 heads (128 cols of C)
for both batches. qT/kT hold both heads stacked on partitions (h0 rows 0:64,
h1 rows 64:128) so score matmuls pair the two heads on disjoint PE row-groups
with no duplicated bias writes. v is produced transposed (w-stationary, wide
moving) then PE-transposed to natural layout via an identity matmul. One A2A
per batch redistributes unnormalized y^T (+ softmax normalizer row) into row
shards; each core then normalizes and computes a disjoint 512-row slice of the
Wo projection. Sends ride the GpSimd queue, unpacks ride the Sync queue so
collective-completion waits never head-block compute-critical queues.
bf16 matmuls, fp32 PSUM accumulation.
"""

import sys

sys.path.insert(0, "/opt/trn_rl_repo")

import numpy as np
import ml_dtypes

import concourse.bass as bass
import concourse.bacc as bacc
import concourse.mybir as mybir
from concourse.tile import TileContext
from concourse.masks import make_identity
from concourse import bass_utils

BF16 = mybir.dt.bfloat16
F32 = mybir.dt.float32
NPBF16 = ml_dtypes.bfloat16

B, T, C, H, D = 2, 2048, 1024, 16, 64
NCORES = 8
HL = H // NCORES          # heads per core = 2
COLS = HL * D             # 128 head-cols per core
KT = C // 128             # 8 contraction k-tiles
NCH = T // 512            # 4 query chunks of 512 per batch
NT = T // 128             # 16 key tiles of 128 per batch
VW = D + 1                # 65: v columns + ones column
SH = 72                   # per-head subshard rows (65 used, pad to 4KB rows)
SH2 = 2 * SH              # both heads stacked per destination shard
ROWS_PER_CORE = B * T // NCORES  # 512 output rows per core

MASK_NEG = -60000.0
SCALE = 1.0 / np.sqrt(np.float32(D))

Exp = mybir.ActivationFunctionType.Exp
Ident = mybir.ActivationFunctionType.Identity


def build_nc():
    nc = bacc.Bacc(
        "TRN2",
        target_bir_lowering=False,
        debug=False,
        enable_asserts=False,
        num_devices=NCORES,
    )
    xT = nc.dram_tensor("xT", [C, B * T], BF16, kind="ExternalInput")
    # weights pre-tiled on host to [128, k-tile blocks] for contiguous DMA
    wq = nc.dram_tensor("wq", [128, KT * COLS], BF16, kind="ExternalInput")
    wk = nc.dram_tensor("wk", [128, KT * COLS], BF16, kind="ExternalInput")
    wv = nc.dram_tensor("wv", [128, KT * COLS], BF16, kind="ExternalInput")
    # wo rows pre-permuted h-major [h, core, 64] and pre-tiled likewise
    wo = nc.dram_tensor("wo", [128, KT * C], BF16, kind="ExternalInput")
    bqk = nc.dram_tensor("bqk", [COLS, 2], F32, kind="ExternalInput")
    bv = nc.dram_tensor("bv", [COLS, 1], F32, kind="ExternalInput")
    bo = nc.dram_tensor("bo", [1, C], BF16, kind="ExternalInput")
    mtri = nc.dram_tensor("mtri", [128, 128], BF16, kind="ExternalInput")
    emat = nc.dram_tensor("emat", [NCORES, C], BF16, kind="ExternalInput")
    # one A2A per batch; dst shard = [h0 y^T+r | pad | h1 y^T+r | pad] x 256 tok
    send = [
        nc.dram_tensor(f"a2a_send{b}", [NCORES * SH2, 256], BF16) for b in range(B)
    ]
    recv = [
        nc.dram_tensor(f"a2a_recv{b}", [NCORES * SH2, 256], BF16) for b in range(B)
    ]
    wsend = nc.dram_tensor("warm_send", [NCORES * 2, 512], BF16)
    wrecv = nc.dram_tensor("warm_recv", [NCORES * 2, 512], BF16)
    out = nc.dram_tensor("out", [ROWS_PER_CORE, C], F32, kind="ExternalOutput")

    add = mybir.AluOpType.add
    mult = mybir.AluOpType.mult

    def stage_a_batch(b, x_sb, psA, psT, cs):
        # q/k projections into transposed, head-stacked layout
        for wsb, dsb, bcol in ((cs["wq"], cs["qT"], 0), (cs["wk"], cs["kT"], 1)):
            for n in range(NCH):
                ps = psA.tile([128, 512], F32, tag="psA", name="psA", bufs=4)
                col = b * T + n * 512
                for k in range(KT):
                    nc.tensor.matmul(
                        ps[:],
                        wsb[:, k * COLS : (k + 1) * COLS],
                        x_sb[k][:, col : col + 512],
                        start=(k == 0),
                        stop=(k == KT - 1),
                    )
                nc.scalar.activation(
                    dsb[:, col : col + 512],
                    ps[:],
                    Ident,
                    bias=cs["bqk"][:, bcol : bcol + 1],
                )
        # v^T (w-stationary, wide moving), then PE-transpose to natural layout
        for n in range(NCH):
            ps = psA.tile([128, 512], F32, tag="psA", name="psA", bufs=4)
            col = b * T + n * 512
            for k in range(KT):
                nc.tensor.matmul(
                    ps[:],
                    cs["wv"][:, k * COLS : (k + 1) * COLS],
                    x_sb[k][:, col : col + 512],
                    start=(k == 0),
                    stop=(k == KT - 1),
                )
            nc.scalar.activation(
                cs["vT"][:, col : col + 512], ps[:], Ident, bias=cs["bv"][:, 0:1]
            )
        for m in range(NT):
            pv = psT.tile([128, 128], F32, tag="psT", name="psT", bufs=2)
            col = b * T + m * 128
            # out = vT_tile^T via moving identity
            nc.tensor.matmul(
                pv[:], cs["vT"][:, col : col + 128], cs["ident"][:],
                start=True, stop=True,
            )
            vi = (b * NT + m) * HL
            nc.vector.tensor_copy(
                out=cs["v"][:, vi : vi + HL, 0:D],
                in_=pv[:, :].rearrange("p (h d) -> p h d", h=HL),
            )

    def scores_chunk(b, n, pts, ptp, psS, cs):
        """Generator: score tiles + exp for query chunk (b, n); h0/h1 paired
        on disjoint PE row-groups within ONE [128,1024] PSUM tile (same-tile
        allocation keeps the pair adjacent through the scheduler, like the
        PE needs for row-group overlap). One exp covers both heads. Appends
        pt (=exp(scaled scores)^T) half-tiles per head; yields per key tile."""
        qT, kT = cs["qT"], cs["kT"]
        qcol = b * T + n * 512
        for m in range(4 * n + 4):
            kcol = b * T + m * 128
            j = m - 4 * n  # >= 0 only for the diagonal block tiles
            ps2 = psS.tile([128, 1024], F32, tag="ps2", name="ps2", bufs=3)
            if j > 0:
                # fully-masked left columns: pre-set PSUM so exp yields 0
                for h in range(HL):
                    nc.vector.memset(
                        ps2[:, h * 512 : h * 512 + j * 128], MASK_NEG
                    )
            for h in range(HL):
                nc.tensor.matmul(
                    ps2[:, h * 512 + max(j, 0) * 128 : (h + 1) * 512],
                    kT[h * D : (h + 1) * D, kcol : kcol + 128],
                    qT[h * D : (h + 1) * D, qcol + max(j, 0) * 128 : qcol + 512],
                    start=True,
                    stop=True,
                )
            if j >= 0:
                # causal mask of the diagonal 128-block
                for h in range(HL):
                    nc.vector.tensor_tensor(
                        ps2[:, h * 512 + j * 128 : h * 512 + (j + 1) * 128],
                        ps2[:, h * 512 + j * 128 : h * 512 + (j + 1) * 128],
                        cs["mtri"][:],
                        add,
                    )
            pt2 = ptp.tile([128, 1024], BF16, tag="pt", name="pt", bufs=28)
            nc.scalar.activation(pt2[:], ps2[:], Exp, scale=float(SCALE))
            pts[0].append(pt2[:, 0:512])
            pts[1].append(pt2[:, 512:1024])
            yield

    def av_chunk(b, n, pts, psY, nrm, cs):
        """Generator: y^T (+ normalizer row) = v_aug^T @ P^T for chunk (b, n),
        then ship unnormalized y^T plus the r row (receive side divides).
        Yields after each matmul so the driver can interleave score pairs."""
        last = 4 * n + 3
        for h in range(HL):
            py = psY.tile([VW, 512], F32, tag="psY", name="psY", bufs=2)
            for m in range(4 * n + 4):
                vi = (b * NT + m) * HL + h
                nc.tensor.matmul(
                    py[:],
                    cs["v"][:, vi : vi + 1, :],
                    pts[h][m][:],
                    start=(m == 0),
                    stop=(m == last),
                )
                if m < last:
                    yield
            yn = nrm.tile([VW, 512], BF16, tag="yn", name="yn", bufs=6)
            nc.vector.tensor_copy(out=yn[:], in_=py[:])
            for p in range(2):
                dst = 2 * n + p
                o = send[b][dst * SH2 + h * SH : dst * SH2 + h * SH + VW, :]
                nc.gpsimd.dma_start(out=o, in_=yn[:, p * 256 : (p + 1) * 256])
            yield

    def a2a(b):
        nc.gpsimd.collective_compute(
            "AllToAll",
            mybir.AluOpType.bypass,
            replica_groups=[list(range(NCORES))],
            ins=[send[b][:]],
            outs=[recv[b][:]],
        )

    def make_inv(cp, r_sb, tag):
        r_f = cp.tile([NCORES, 256], F32, tag=f"rf{tag}", name=f"rf{tag}")
        nc.vector.tensor_copy(out=r_f[:], in_=r_sb[:])
        invf = cp.tile([NCORES, 256], F32, tag=f"invf{tag}", name=f"invf{tag}")
        nc.vector.reciprocal_approx_fast(out=invf[:], in_=r_f[:])
        inv = cp.tile([NCORES, 256], BF16, tag=f"inv{tag}", name=f"inv{tag}")
        nc.vector.tensor_copy(out=inv[:], in_=invf[:])
        return inv

    def stage_c_batch(b, cp, psC, cs, y_sb, yn_sb):
        """After a2a(b): unpack, normalize, Wo rows 2b,2b+1, store."""
        c0 = b * 256
        rv = recv[b].rearrange("(k p dr) c -> p dr k c", k=4, p=2)
        for p in range(2):
            for hh, k0 in ((0, 0), (SH, 4)):
                nc.sync.dma_start(
                    out=y_sb[p * D : p * D + D, k0 * 512 : (k0 + 4) * 512].rearrange(
                        "d (k c) -> d k c", k=4
                    )[:, :, c0 : c0 + 256],
                    in_=rv[p, hh : hh + D, :, :],
                )
        rr = recv[b].rearrange("(s dr) c -> s dr c", s=NCORES)
        r0_sb = cp.tile([NCORES, 256], BF16, tag=f"rsb0{b}", name=f"rsb0{b}")
        r1_sb = cp.tile([NCORES, 256], BF16, tag=f"rsb1{b}", name=f"rsb1{b}")
        nc.sync.dma_start(
            out=r0_sb[:].rearrange("s (o c) -> s o c", o=1), in_=rr[:, D : D + 1, :]
        )
        nc.sync.dma_start(
            out=r1_sb[:].rearrange("s (o c) -> s o c", o=1),
            in_=rr[:, SH + D : SH + D + 1, :],
        )
        inv0 = make_inv(cp, r0_sb, f"0{b}")
        inv1 = make_inv(cp, r1_sb, f"1{b}")
        for k in range(8):
            inv = inv0 if k < 4 else inv1
            pb = psC.tile([128, 256], F32, tag="psB", name="psB", bufs=2)
            nc.tensor.matmul(
                pb[:],
                cs["emat"][:, (k % 4) * 128 : (k % 4 + 1) * 128],
                inv[:],
                start=True,
                stop=True,
            )
            nc.vector.tensor_tensor(
                yn_sb[:, k * 512 + c0 : k * 512 + c0 + 256],
                y_sb[:, k * 512 + c0 : k * 512 + c0 + 256],
                pb[:],
                mult,
            )
        for r in (2 * b, 2 * b + 1):
            for o in range(C // 512):
                pc = psC.tile([128, 512], F32, tag="psC", name="psC", bufs=4)
                for k in range(KT):
                    nc.tensor.matmul(
                        pc[:],
                        yn_sb[:, k * 512 + r * 128 : k * 512 + r * 128 + 128],
                        cs["wo"][:, k * C + o * 512 : k * C + (o + 1) * 512],
                        start=(k == 0),
                        stop=False,
                    )
                nc.tensor.matmul(
                    pc[:],
                    cs["ones"][0:1, :],
                    cs["bo"][0:1, o * 512 : (o + 1) * 512],
                    start=False,
                    stop=True,
                )
                osb = cp.tile([128, 512], F32, tag="osb", name="osb", bufs=3)
                nc.vector.tensor_copy(out=osb[:], in_=pc[:])
                nc.scalar.dma_start(
                    out=out[r * 128 : (r + 1) * 128, o * 512 : (o + 1) * 512],
                    in_=osb[:],
                )

    with TileContext(nc) as tc:
        with tc.tile_pool(name="persist", bufs=1) as pp:
            cs = {}
            # warmup collective first: absorbs the ~11us first-trigger latency
            nc.gpsimd.collective_compute(
                "AllToAll",
                mybir.AluOpType.bypass,
                replica_groups=[list(range(NCORES))],
                ins=[wsend[:]],
                outs=[wrecv[:]],
            )
            # tiny constants FIRST: a late-landing constant can head-block the
            # PE queue (first scheduled LDWEIGHTS waits on it)
            cs["ones"] = pp.tile([1, 128], BF16, tag="ones", name="ones")
            nc.vector.memset(cs["ones"][:], 1.0)
            cs["bqk"] = pp.tile([COLS, 2], F32, tag="bqk", name="bqk")
            nc.sync.dma_start(out=cs["bqk"][:], in_=bqk[:])
            cs["bv"] = pp.tile([COLS, 1], F32, tag="bv", name="bv")
            nc.sync.dma_start(out=cs["bv"][:], in_=bv[:])
            cs["bo"] = pp.tile([1, C], BF16, tag="bo", name="bo")
            nc.scalar.dma_start(out=cs["bo"][:], in_=bo[:])
            cs["mtri"] = pp.tile([128, 128], BF16, tag="mtri", name="mtri")
            nc.scalar.dma_start(out=cs["mtri"][:], in_=mtri[:])
            cs["emat"] = pp.tile([NCORES, C], BF16, tag="emat", name="emat")
            nc.gpsimd.dma_start(out=cs["emat"][:], in_=emat[:])
            cs["ident"] = pp.tile([128, 128], BF16, tag="ident", name="ident")
            make_identity(nc, cs["ident"][:])

            # weights next (needed with first x tiles)
            cs["wq"] = pp.tile([128, KT * COLS], BF16, tag="wq", name="wq")
            cs["wk"] = pp.tile([128, KT * COLS], BF16, tag="wk", name="wk")
            cs["wv"] = pp.tile([128, KT * COLS], BF16, tag="wv", name="wv")
            nc.sync.dma_start(out=cs["wq"][:], in_=wq[:])
            nc.scalar.dma_start(out=cs["wk"][:], in_=wk[:])
            nc.gpsimd.dma_start(out=cs["wv"][:], in_=wv[:])

            with tc.tile_pool(name="xp", bufs=1) as xp:
                # x k-tiles split into 512-col chunks over 4 engine DMA queues
                # (not tensor: DMA issues there would head-block the first
                # matmuls), first-consumed columns first
                x_sb = [
                    xp.tile([128, B * T], BF16, tag=f"x{k}", name=f"x{k}")
                    for k in range(KT)
                ]
                xq = [nc.sync, nc.scalar, nc.gpsimd]
                qi = 0
                for c in range(B * T // 512):
                    for k in range(KT):
                        xq[qi % 3].dma_start(
                            out=x_sb[k][:, c * 512 : (c + 1) * 512],
                            in_=xT[k * 128 : (k + 1) * 128, c * 512 : (c + 1) * 512],
                        )
                        qi += 1

                cs["qT"] = pp.tile([128, B * T], BF16, tag="qT", name="qT")
                cs["kT"] = pp.tile([128, B * T], BF16, tag="kT", name="kT")
                cs["vT"] = pp.tile([128, B * T], BF16, tag="vT", name="vT")
                cs["v"] = pp.tile([128, B * NT * HL, VW], BF16, tag="v", name="v")
                nc.gpsimd.memset(cs["v"][:], 1.0)  # presets the ones columns

                # wo loaded last (not needed until stage C)
                cs["wo"] = pp.tile([128, KT * C], BF16, tag="wo", name="wo")
                nc.scalar.dma_start(out=cs["wo"][:], in_=wo[:])

                with tc.tile_pool(name="psA", bufs=1, space="PSUM") as psA, \
                     tc.tile_pool(name="psT", bufs=1, space="PSUM") as psT:
                    for b in range(B):
                        stage_a_batch(b, x_sb, psA, psT, cs)

            with tc.tile_pool(name="pt", bufs=1) as ptp, tc.tile_pool(
                name="psS", bufs=1, space="PSUM"
            ) as psS, tc.tile_pool(
                name="psY", bufs=1, space="PSUM"
            ) as psY, tc.tile_pool(
                name="nrm", bufs=1
            ) as nrm:
                # software pipeline: interleave score pairs of chunk u with the
                # AV matmuls of chunk u-1 at instruction level, so the
                # scalar-engine exps (slower than paired score production)
                # overlap the AV matmul stream instead of serializing
                prev_pts = None
                for u in range(B * NCH + 1):
                    sg = None
                    if u < B * NCH:
                        cur_pts = ([], [])
                        sg = scores_chunk(u // NCH, u % NCH, cur_pts, ptp, psS, cs)
                    ag = None
                    if u > 0:
                        pb, pn = (u - 1) // NCH, (u - 1) % NCH
                        ag = av_chunk(pb, pn, prev_pts, psY, nrm, cs)
                    if sg is not None and ag is not None:
                        s_steps = 4 * (u % NCH) + 4
                        a_steps = 2 * (4 * pn + 4)
                        acc = 0.0
                        for _ in range(s_steps):
                            next(sg, None)
                            acc += a_steps / s_steps
                            while acc >= 1.0:
                                next(ag, None)
                                acc -= 1.0
                    if sg is not None:
                        for _ in sg:
                            pass
                    if ag is not None:
                        for _ in ag:
                            pass
                        if pn == NCH - 1:
                            a2a(pb)
                    prev_pts = cur_pts
            with tc.tile_pool(name="cp", bufs=1) as cp, tc.tile_pool(
                name="psC", bufs=1, space="PSUM"
            ) as psC:
                y_sb = cp.tile([128, 8 * 512], BF16, tag="ysb", name="ysb")
                yn_sb = cp.tile([128, 8 * 512], BF16, tag="ynsb", name="ynsb")
                for b in range(B):
                    stage_c_batch(b, cp, psC, cs, y_sb, yn_sb)
    nc.compile()
    return nc


def make_in_maps(x, mask, Wq, bq, Wk, bk, Wv, bv, Wo, bo):
    xT = np.ascontiguousarray(
        x.astype(np.float32).transpose(2, 0, 1).reshape(C, B * T)
    ).astype(NPBF16)
    mtri = np.where(
        np.arange(128)[:, None] > np.arange(128)[None, :], MASK_NEG, 0.0
    ).astype(NPBF16)
    # Wo rows permuted h-major: new row order = [core0 h0 d0..63, core1 h0, ...,
    # core7 h0, core0 h1, ..., core7 h1]
    perm = np.concatenate(
        [
            np.arange(c * COLS + h * D, c * COLS + h * D + D)
            for h in range(HL)
            for c in range(NCORES)
        ]
    )
    def pretile(w):
        # [C, width] -> [128, KT*width] with k-tile blocks along free axis
        width = w.shape[1]
        return np.ascontiguousarray(
            w.reshape(KT, 128, width).transpose(1, 0, 2).reshape(128, KT * width)
        )
    wo_b = pretile(Wo[perm].astype(NPBF16))
    # emat[s, c] = 1 iff shard s supplies the softmax normalizer for y-column
    # position c of the per-head k-chunk layout (chunk k = shards 2k, 2k+1)
    cols = np.arange(C)
    emat_np = (
        np.arange(NCORES)[:, None] == (2 * (cols // 128) + (cols % 128) // 64)
    ).astype(NPBF16)
    bo_b = bo.reshape(1, C).astype(NPBF16)
    in_maps = []
    for c in range(NCORES):
        cslice = slice(c * COLS, (c + 1) * COLS)
        in_maps.append(
            {
                "xT": xT,
                "wq": pretile(Wq[:, cslice].astype(NPBF16)),
                "wk": pretile(Wk[:, cslice].astype(NPBF16)),
                "wv": pretile(Wv[:, cslice].astype(NPBF16)),
                "wo": wo_b,
                "bqk": np.stack([bq[cslice], bk[cslice]], axis=1).astype(
                    np.float32
                ),
                "bv": bv[cslice].reshape(COLS, 1).astype(np.float32),
                "bo": bo_b,
                "mtri": mtri,
                "emat": emat_np,
            }
        )
    return in_maps


_CACHED_NC = None


def run(inputs, trace=False, **kw):
    global _CACHED_NC
    if _CACHED_NC is None:
        _CACHED_NC = build_nc()
    in_maps = make_in_maps(**inputs)
    res = bass_utils.run_bass_kernel_spmd(
        _CACHED_NC, in_maps, core_ids=list(range(NCORES)), trace=trace, **kw
    )
    outs = [np.asarray(res.results[c]["out"]) for c in range(NCORES)]
    full = np.empty((B, T, C), np.float32)
    for j in range(NCORES):
        full[0, 256 * j : 256 * (j + 1)] = outs[j][0:256]
        full[1, 256 * j : 256 * (j + 1)] = outs[j][256:512]
    return full, res


def kernel(**inputs):
    full, _ = run(inputs, trace=False)
    return full


# revision 14
# speedup vs baseline: 1.2816x; 1.0337x over previous
"""Causal self-attention, tensor-parallel over heads across 8 TRN2 NeuronCores.

B=2, T=2048, C=1024, H=16 heads, D=64. Each core owns 2 heads (128 cols of C)
for both batches. qT/kT hold both heads stacked on partitions (h0 rows 0:64,
h1 rows 64:128) so score matmuls pair the two heads on disjoint PE row-groups
with no duplicated bias writes. v is produced transposed (w-stationary, wide
moving) then PE-transposed to natural layout via an identity matmul. One A2A
per batch redistributes unnormalized y^T (+ softmax normalizer row) into row
shards; each core then normalizes and computes a disjoint 512-row slice of the
Wo projection. Sends ride the GpSimd queue, unpacks ride the Sync queue so
collective-completion waits never head-block compute-critical queues.
bf16 matmuls, fp32 PSUM accumulation.
"""

import sys

sys.path.insert(0, "/opt/trn_rl_repo")

import numpy as np
import ml_dtypes

import concourse.bass as bass
import concourse.bacc as bacc
import concourse.mybir as mybir
from concourse.tile import TileContext
from concourse.masks import make_identity
from concourse import bass_utils

BF16 = mybir.dt.bfloat16
F32 = mybir.dt.float32
NPBF16 = ml_dtypes.bfloat16

B, T, C, H, D = 2, 2048, 1024, 16, 64
NCORES = 8
HL = H // NCORES          # heads per core = 2
COLS = HL * D             # 128 head-cols per core
KT = C // 128             # 8 contraction k-tiles
NCH = T // 512            # 4 query chunks of 512 per batch
NT = T // 128             # 16 key tiles of 128 per batch
VW = D + 1                # 65: v columns + ones column
SH = 72                   # per-head subshard rows (65 used, pad to 4KB rows)
SH2 = 2 * SH              # both heads stacked per destination shard
ROWS_PER_CORE = B * T // NCORES  # 512 output rows per core

MASK_NEG = -60000.0
SCALE = 1.0 / np.sqrt(np.float32(D))

Exp = mybir.ActivationFunctionType.Exp
Ident = mybir.ActivationFunctionType.Identity


def build_nc():
    nc = bacc.Bacc(
        "TRN2",
        target_bir_lowering=False,
        debug=False,
        enable_asserts=False,
        num_devices=NCORES,
    )
    xT = nc.dram_tensor("xT", [C, B * T], BF16, kind="ExternalInput")
    # weights pre-tiled on host to [128, k-tile blocks] for contiguous DMA
    wq = nc.dram_tensor("wq", [128, KT * COLS], BF16, kind="ExternalInput")
    wk = nc.dram_tensor("wk", [128, KT * COLS], BF16, kind="ExternalInput")
    wv = nc.dram_tensor("wv", [128, KT * COLS], BF16, kind="ExternalInput")
    # wo rows pre-permuted h-major [h, core, 64] and pre-tiled likewise
    wo = nc.dram_tensor("wo", [128, KT * C], BF16, kind="ExternalInput")
    bqk = nc.dram_tensor("bqk", [COLS, 2], F32, kind="ExternalInput")
    bv = nc.dram_tensor("bv", [COLS, 1], F32, kind="ExternalInput")
    bo = nc.dram_tensor("bo", [1, C], BF16, kind="ExternalInput")
    mtri = nc.dram_tensor("mtri", [128, 128], BF16, kind="ExternalInput")
    emat = nc.dram_tensor("emat", [NCORES, C], BF16, kind="ExternalInput")
    # one A2A per batch; dst shard = [h0 y^T+r | pad | h1 y^T+r | pad] x 256 tok
    send = [
        nc.dram_tensor(f"a2a_send{b}", [NCORES * SH2, 256], BF16) for b in range(B)
    ]
    recv = [
        nc.dram_tensor(f"a2a_recv{b}", [NCORES * SH2, 256], BF16) for b in range(B)
    ]
    wsend = nc.dram_tensor("warm_send", [NCORES * 2, 512], BF16)
    wrecv = nc.dram_tensor("warm_recv", [NCORES * 2, 512], BF16)
    out = nc.dram_tensor("out", [ROWS_PER_CORE, C], F32, kind="ExternalOutput")

    add = mybir.AluOpType.add
    mult = mybir.AluOpType.mult

    def stage_a_batch(b, x_sb, psA, psT, cs):
        # q/k projections into transposed, head-stacked layout
        for wsb, dsb, bcol in ((cs["wq"], cs["qT"], 0), (cs["wk"], cs["kT"], 1)):
            for n in range(NCH):
                ps = psA.tile([128, 512], F32, tag="psA", name="psA", bufs=4)
                col = b * T + n * 512
                for k in range(KT):
                    nc.tensor.matmul(
                        ps[:],
                        wsb[:, k * COLS : (k + 1) * COLS],
                        x_sb[k][:, col : col + 512],
                        start=(k == 0),
                        stop=(k == KT - 1),
                    )
                nc.scalar.activation(
                    dsb[:, col : col + 512],
                    ps[:],
                    Ident,
                    bias=cs["bqk"][:, bcol : bcol + 1],
                )
        # v^T (w-stationary, wide moving), then PE-transpose to natural layout
        for n in range(NCH):
            ps = psA.tile([128, 512], F32, tag="psA", name="psA", bufs=4)
            col = b * T + n * 512
            for k in range(KT):
                nc.tensor.matmul(
                    ps[:],
                    cs["wv"][:, k * COLS : (k + 1) * COLS],
                    x_sb[k][:, col : col + 512],
                    start=(k == 0),
                    stop=(k == KT - 1),
                )
            nc.scalar.activation(
                cs["vT"][:, col : col + 512], ps[:], Ident, bias=cs["bv"][:, 0:1]
            )
        for m in range(NT):
            pv = psT.tile([128, 128], F32, tag="psT", name="psT", bufs=2)
            col = b * T + m * 128
            # out = vT_tile^T via moving identity
            nc.tensor.matmul(
                pv[:], cs["vT"][:, col : col + 128], cs["ident"][:],
                start=True, stop=True,
            )
            vi = (b * NT + m) * HL
            nc.vector.tensor_copy(
                out=cs["v"][:, vi : vi + HL, 0:D],
                in_=pv[:, :].rearrange("p (h d) -> p h d", h=HL),
            )

    def scores_chunk(b, n, pts, ptp, psS, cs):
        """Generator: score tiles + exp for query chunk (b, n); h0/h1 paired
        on disjoint PE row-groups within ONE [128,1024] PSUM tile (same-tile
        allocation keeps the pair adjacent through the scheduler, like the
        PE needs for row-group overlap). One exp covers both heads. Appends
        pt (=exp(scaled scores)^T) half-tiles per head; yields per key tile."""
        qT, kT = cs["qT"], cs["kT"]
        qcol = b * T + n * 512
        for m in range(4 * n + 4):
            kcol = b * T + m * 128
            j = m - 4 * n  # >= 0 only for the diagonal block tiles
            ps2 = psS.tile([128, 1024], F32, tag="ps2", name="ps2", bufs=3)
            if j > 0:
                # fully-masked left columns: pre-set PSUM so exp yields 0
                for h in range(HL):
                    nc.vector.memset(
                        ps2[:, h * 512 : h * 512 + j * 128], MASK_NEG
                    )
            for h in range(HL):
                nc.tensor.matmul(
                    ps2[:, h * 512 + max(j, 0) * 128 : (h + 1) * 512],
                    kT[h * D : (h + 1) * D, kcol : kcol + 128],
                    qT[h * D : (h + 1) * D, qcol + max(j, 0) * 128 : qcol + 512],
                    start=True,
                    stop=True,
                )
            if j >= 0:
                # causal mask of the diagonal 128-block
                for h in range(HL):
                    nc.vector.tensor_tensor(
                        ps2[:, h * 512 + j * 128 : h * 512 + (j + 1) * 128],
                        ps2[:, h * 512 + j * 128 : h * 512 + (j + 1) * 128],
                        cs["mtri"][:],
                        add,
                    )
            pt2 = ptp.tile([128, 1024], BF16, tag="pt", name="pt", bufs=28)
            nc.scalar.activation(pt2[:], ps2[:], Exp, scale=float(SCALE))
            pts[0].append(pt2[:, 0:512])
            pts[1].append(pt2[:, 512:1024])
            yield

    def av_chunk(b, n, pts, psY, nrm, cs):
        """Generator: y^T (+ normalizer row) = v_aug^T @ P^T for chunk (b, n),
        then ship unnormalized y^T plus the r row (receive side divides).
        Yields after each matmul so the driver can interleave score pairs."""
        last = 4 * n + 3
        for h in range(HL):
            py = psY.tile([VW, 512], F32, tag="psY", name="psY", bufs=2)
            for m in range(4 * n + 4):
                vi = (b * NT + m) * HL + h
                nc.tensor.matmul(
                    py[:],
                    cs["v"][:, vi : vi + 1, :],
                    pts[h][m][:],
                    start=(m == 0),
                    stop=(m == last),
                )
                if m < last:
                    yield
            yn = nrm.tile([VW, 512], BF16, tag="yn", name="yn", bufs=6)
            nc.vector.tensor_copy(out=yn[:], in_=py[:])
            for p in range(2):
                dst = 2 * n + p
                o = send[b][dst * SH2 + h * SH : dst * SH2 + h * SH + VW, :]
                nc.gpsimd.dma_start(out=o, in_=yn[:, p * 256 : (p + 1) * 256])
            yield

    def a2a(b):
        nc.gpsimd.collective_compute(
            "AllToAll",
            mybir.AluOpType.bypass,
            replica_groups=[list(range(NCORES))],
            ins=[send[b][:]],
            outs=[recv[b][:]],
        )

    def make_inv(cp, r_sb, tag):
        r_f = cp.tile([NCORES, 256], F32, tag=f"rf{tag}", name=f"rf{tag}")
        nc.vector.tensor_copy(out=r_f[:], in_=r_sb[:])
        invf = cp.tile([NCORES, 256], F32, tag=f"invf{tag}", name=f"invf{tag}")
        nc.vector.reciprocal_approx_fast(out=invf[:], in_=r_f[:])
        inv = cp.tile([NCORES, 256], BF16, tag=f"inv{tag}", name=f"inv{tag}")
        nc.vector.tensor_copy(out=inv[:], in_=invf[:])
        return inv

    def stage_c_batch(b, cp, psC, cs, y_sb, yn_sb):
        """After a2a(b): unpack, normalize, Wo rows 2b,2b+1, store."""
        c0 = b * 256
        rv = recv[b].rearrange("(k p dr) c -> p dr k c", k=4, p=2)
        rr = recv[b].rearrange("(s dr) c -> s dr c", s=NCORES)
        r0_sb = cp.tile([NCORES, 256], BF16, tag=f"rsb0{b}", name=f"rsb0{b}")
        r1_sb = cp.tile([NCORES, 256], BF16, tag=f"rsb1{b}", name=f"rsb1{b}")
        # unpacks spread across the three DMA-capable queues (all idle here)
        # so the post-collective critical path is one strided DMA, not six
        nc.sync.dma_start(
            out=r0_sb[:].rearrange("s (o c) -> s o c", o=1), in_=rr[:, D : D + 1, :]
        )
        nc.scalar.dma_start(
            out=r1_sb[:].rearrange("s (o c) -> s o c", o=1),
            in_=rr[:, SH + D : SH + D + 1, :],
        )
        uq = [nc.gpsimd, nc.sync, nc.scalar, nc.gpsimd]
        for p in range(2):
            for i, (hh, k0) in enumerate(((0, 0), (SH, 4))):
                uq[2 * p + i].dma_start(
                    out=y_sb[p * D : p * D + D, k0 * 512 : (k0 + 4) * 512].rearrange(
                        "d (k c) -> d k c", k=4
                    )[:, :, c0 : c0 + 256],
                    in_=rv[p, hh : hh + D, :, :],
                )
        inv0 = make_inv(cp, r0_sb, f"0{b}")
        inv1 = make_inv(cp, r1_sb, f"1{b}")
        for k in range(8):
            inv = inv0 if k < 4 else inv1
            pb = psC.tile([128, 256], F32, tag="psB", name="psB", bufs=2)
            nc.tensor.matmul(
                pb[:],
                cs["emat"][:, (k % 4) * 128 : (k % 4 + 1) * 128],
                inv[:],
                start=True,
                stop=True,
            )
            nc.vector.tensor_tensor(
                yn_sb[:, k * 512 + c0 : k * 512 + c0 + 256],
                y_sb[:, k * 512 + c0 : k * 512 + c0 + 256],
                pb[:],
                mult,
            )
        for r in (2 * b, 2 * b + 1):
            for o in range(C // 512):
                pc = psC.tile([128, 512], F32, tag="psC", name="psC", bufs=4)
                for k in range(KT):
                    nc.tensor.matmul(
                        pc[:],
                        yn_sb[:, k * 512 + r * 128 : k * 512 + r * 128 + 128],
                        cs["wo"][:, k * C + o * 512 : k * C + (o + 1) * 512],
                        start=(k == 0),
                        stop=False,
                    )
                nc.tensor.matmul(
                    pc[:],
                    cs["ones"][0:1, :],
                    cs["bo"][0:1, o * 512 : (o + 1) * 512],
                    start=False,
                    stop=True,
                )
                osb = cp.tile([128, 512], F32, tag="osb", name="osb", bufs=3)
                nc.vector.tensor_copy(out=osb[:], in_=pc[:])
                # split the store across two queues: the final tiles are on
                # the critical tail, and one 256KB DMA is ~2x slower
                for ho, q in ((0, nc.scalar), (1, nc.gpsimd)):
                    q.dma_start(
                        out=out[
                            r * 128 : (r + 1) * 128,
                            o * 512 + ho * 256 : o * 512 + (ho + 1) * 256,
                        ],
                        in_=osb[:, ho * 256 : (ho + 1) * 256],
                    )

    with TileContext(nc) as tc:
        with tc.tile_pool(name="persist", bufs=1) as pp:
            cs = {}
            # warmup collective first: absorbs the ~11us first-trigger latency
            nc.gpsimd.collective_compute(
                "AllToAll",
                mybir.AluOpType.bypass,
                replica_groups=[list(range(NCORES))],
                ins=[wsend[:]],
                outs=[wrecv[:]],
            )
            # tiny constants FIRST: a late-landing constant can head-block the
            # PE queue (first scheduled LDWEIGHTS waits on it)
            cs["ones"] = pp.tile([1, 128], BF16, tag="ones", name="ones")
            nc.vector.memset(cs["ones"][:], 1.0)
            cs["bqk"] = pp.tile([COLS, 2], F32, tag="bqk", name="bqk")
            nc.sync.dma_start(out=cs["bqk"][:], in_=bqk[:])
            cs["bv"] = pp.tile([COLS, 1], F32, tag="bv", name="bv")
            nc.sync.dma_start(out=cs["bv"][:], in_=bv[:])
            cs["bo"] = pp.tile([1, C], BF16, tag="bo", name="bo")
            nc.scalar.dma_start(out=cs["bo"][:], in_=bo[:])
            cs["mtri"] = pp.tile([128, 128], BF16, tag="mtri", name="mtri")
            nc.scalar.dma_start(out=cs["mtri"][:], in_=mtri[:])
            cs["emat"] = pp.tile([NCORES, C], BF16, tag="emat", name="emat")
            nc.gpsimd.dma_start(out=cs["emat"][:], in_=emat[:])
            cs["ident"] = pp.tile([128, 128], BF16, tag="ident", name="ident")
            make_identity(nc, cs["ident"][:])

            # weights next (needed with first x tiles)
            cs["wq"] = pp.tile([128, KT * COLS], BF16, tag="wq", name="wq")
            cs["wk"] = pp.tile([128, KT * COLS], BF16, tag="wk", name="wk")
            cs["wv"] = pp.tile([128, KT * COLS], BF16, tag="wv", name="wv")
            nc.sync.dma_start(out=cs["wq"][:], in_=wq[:])
            nc.scalar.dma_start(out=cs["wk"][:], in_=wk[:])
            nc.gpsimd.dma_start(out=cs["wv"][:], in_=wv[:])

            with tc.tile_pool(name="xp", bufs=1) as xp:
                # x k-tiles split into 512-col chunks, batch-0 columns first.
                # batch-1 chunks stay OFF the scalar queue: its DMA-issue
                # backpressure would head-block the q/k PSUM->SBUF copies and
                # exps that keep stage B fed.
                x_sb = [
                    xp.tile([128, B * T], BF16, tag=f"x{k}", name=f"x{k}")
                    for k in range(KT)
                ]

                def load_x(cols, queues):
                    qi = 0
                    for c in cols:
                        for k in range(KT):
                            queues[qi % len(queues)].dma_start(
                                out=x_sb[k][:, c * 512 : (c + 1) * 512],
                                in_=xT[
                                    k * 128 : (k + 1) * 128, c * 512 : (c + 1) * 512
                                ],
                            )
                            qi += 1

                cs["qT"] = pp.tile([128, B * T], BF16, tag="qT", name="qT")
                cs["kT"] = pp.tile([128, B * T], BF16, tag="kT", name="kT")
                cs["vT"] = pp.tile([128, B * T], BF16, tag="vT", name="vT")
                cs["v"] = pp.tile([128, B * NT * HL, VW], BF16, tag="v", name="v")
                nc.vector.memset(cs["v"][:], 1.0)  # presets the ones columns

                with tc.tile_pool(name="psA", bufs=1, space="PSUM") as psA, \
                     tc.tile_pool(name="psT", bufs=1, space="PSUM") as psT:
                    load_x(range(4), [nc.sync, nc.scalar, nc.gpsimd])
                    stage_a_batch(0, x_sb, psA, psT, cs)
                    load_x(range(4, 8), [nc.sync, nc.gpsimd])
                    stage_a_batch(1, x_sb, psA, psT, cs)

                # wo issued late on scalar: behind the stage-A copies, well
                # before stage C needs it
                cs["wo"] = pp.tile([128, KT * C], BF16, tag="wo", name="wo")
                nc.scalar.dma_start(out=cs["wo"][:], in_=wo[:])

            with tc.tile_pool(name="pt", bufs=1) as ptp, tc.tile_pool(
                name="psS", bufs=1, space="PSUM"
            ) as psS, tc.tile_pool(
                name="psY", bufs=1, space="PSUM"
            ) as psY, tc.tile_pool(
                name="nrm", bufs=1
            ) as nrm:
                # software pipeline: interleave score pairs of chunk u with the
                # AV matmuls of chunk u-1 at instruction level, so the
                # scalar-engine exps (slower than paired score production)
                # overlap the AV matmul stream instead of serializing
                prev_pts = None
                for u in range(B * NCH + 1):
                    sg = None
                    if u < B * NCH:
                        cur_pts = ([], [])
                        sg = scores_chunk(u // NCH, u % NCH, cur_pts, ptp, psS, cs)
                    ag = None
                    if u > 0:
                        pb, pn = (u - 1) // NCH, (u - 1) % NCH
                        ag = av_chunk(pb, pn, prev_pts, psY, nrm, cs)
                    if sg is not None and ag is not None:
                        s_steps = 4 * (u % NCH) + 4
                        a_steps = 2 * (4 * pn + 4)
                        acc = 0.0
                        for _ in range(s_steps):
                            next(sg, None)
                            acc += a_steps / s_steps
                            while acc >= 1.0:
                                next(ag, None)
                                acc -= 1.0
                    if sg is not None:
                        for _ in sg:
                            pass
                    if ag is not None:
                        for _ in ag:
                            pass
                        if pn == NCH - 1:
                            a2a(pb)
                    prev_pts = cur_pts
            with tc.tile_pool(name="cp", bufs=1) as cp, tc.tile_pool(
                name="psC", bufs=1, space="PSUM"
            ) as psC:
                y_sb = cp.tile([128, 8 * 512], BF16, tag="ysb", name="ysb")
                yn_sb = cp.tile([128, 8 * 512], BF16, tag="ynsb", name="ynsb")
                for b in range(B):
                    stage_c_batch(b, cp, psC, cs, y_sb, yn_sb)
    nc.compile()
    return nc


def make_in_maps(x, mask, Wq, bq, Wk, bk, Wv, bv, Wo, bo):
    xT = np.ascontiguousarray(
        x.astype(np.float32).transpose(2, 0, 1).reshape(C, B * T)
    ).astype(NPBF16)
    mtri = np.where(
        np.arange(128)[:, None] > np.arange(128)[None, :], MASK_NEG, 0.0
    ).astype(NPBF16)
    # Wo rows permuted h-major: new row order = [core0 h0 d0..63, core1 h0, ...,
    # core7 h0, core0 h1, ..., core7 h1]
    perm = np.concatenate(
        [
            np.arange(c * COLS + h * D, c * COLS + h * D + D)
            for h in range(HL)
            for c in range(NCORES)
        ]
    )
    def pretile(w):
        # [C, width] -> [128, KT*width] with k-tile blocks along free axis
        width = w.shape[1]
        return np.ascontiguousarray(
            w.reshape(KT, 128, width).transpose(1, 0, 2).reshape(128, KT * width)
        )
    wo_b = pretile(Wo[perm].astype(NPBF16))
    # emat[s, c] = 1 iff shard s supplies the softmax normalizer for y-column
    # position c of the per-head k-chunk layout (chunk k = shards 2k, 2k+1)
    cols = np.arange(C)
    emat_np = (
        np.arange(NCORES)[:, None] == (2 * (cols // 128) + (cols % 128) // 64)
    ).astype(NPBF16)
    bo_b = bo.reshape(1, C).astype(NPBF16)
    in_maps = []
    for c in range(NCORES):
        cslice = slice(c * COLS, (c + 1) * COLS)
        in_maps.append(
            {
                "xT": xT,
                "wq": pretile(Wq[:, cslice].astype(NPBF16)),
                "wk": pretile(Wk[:, cslice].astype(NPBF16)),
                "wv": pretile(Wv[:, cslice].astype(NPBF16)),
                "wo": wo_b,
                "bqk": np.stack([bq[cslice], bk[cslice]], axis=1).astype(
                    np.float32
                ),
                "bv": bv[cslice].reshape(COLS, 1).astype(np.float32),
                "bo": bo_b,
                "mtri": mtri,
                "emat": emat_np,
            }
        )
    return in_maps


_CACHED_NC = None


def run(inputs, trace=False, **kw):
    global _CACHED_NC
    if _CACHED_NC is None:
        _CACHED_NC = build_nc()
    in_maps = make_in_maps(**inputs)
    res = bass_utils.run_bass_kernel_spmd(
        _CACHED_NC, in_maps, core_ids=list(range(NCORES)), trace=trace, **kw
    )
    outs = [np.asarray(res.results[c]["out"]) for c in range(NCORES)]
    full = np.empty((B, T, C), np.float32)
    for j in range(NCORES):
        full[0, 256 * j : 256 * (j + 1)] = outs[j][0:256]
        full[1, 256 * j : 256 * (j + 1)] = outs[j][256:512]
    return full, res


def kernel(**inputs):
    full, _ = run(inputs, trace=False)
    return full


# revision 17
# speedup vs baseline: 1.3211x; 1.0308x over previous
"""Causal self-attention, tensor-parallel over heads across 8 TRN2 NeuronCores.

B=2, T=2048, C=1024, H=16 heads, D=64. Each core owns 2 heads (128 cols of C)
for both batches. qT/kT hold both heads stacked on partitions (h0 rows 0:64,
h1 rows 64:128) so score matmuls pair the two heads on disjoint PE row-groups
with no duplicated bias writes. v is produced transposed (w-stationary, wide
moving) then PE-transposed to natural layout via an identity matmul. One A2A
per batch redistributes unnormalized y^T (+ softmax normalizer row) into row
shards; each core then normalizes and computes a disjoint 512-row slice of the
Wo projection. Sends ride the GpSimd queue, unpacks ride the Sync queue so
collective-completion waits never head-block compute-critical queues.
bf16 matmuls, fp32 PSUM accumulation.
"""

import sys

sys.path.insert(0, "/opt/trn_rl_repo")

import numpy as np
import ml_dtypes

import concourse.bass as bass
import concourse.bacc as bacc
import concourse.mybir as mybir
from concourse.tile import TileContext
from concourse.masks import make_identity
from concourse import bass_utils

BF16 = mybir.dt.bfloat16
F32 = mybir.dt.float32
NPBF16 = ml_dtypes.bfloat16

B, T, C, H, D = 2, 2048, 1024, 16, 64
NCORES = 8
HL = H // NCORES          # heads per core = 2
COLS = HL * D             # 128 head-cols per core
KT = C // 128             # 8 contraction k-tiles
NCH = T // 512            # 4 query chunks of 512 per batch
NT = T // 128             # 16 key tiles of 128 per batch
VW = D + 1                # 65: v columns + ones column
SH = 72                   # per-head subshard rows (65 used, pad to 4KB rows)
SH2 = 2 * SH              # both heads stacked per destination shard
ROWS_PER_CORE = B * T // NCORES  # 512 output rows per core

MASK_NEG = -60000.0
SCALE = 1.0 / np.sqrt(np.float32(D))

Exp = mybir.ActivationFunctionType.Exp
Ident = mybir.ActivationFunctionType.Identity


def build_nc():
    nc = bacc.Bacc(
        "TRN2",
        target_bir_lowering=False,
        debug=False,
        enable_asserts=False,
        num_devices=NCORES,
    )
    xT = nc.dram_tensor("xT", [C, B * T], BF16, kind="ExternalInput")
    # weights pre-tiled on host to [128, k-tile blocks] for contiguous DMA
    wq = nc.dram_tensor("wq", [128, KT * COLS], BF16, kind="ExternalInput")
    wk = nc.dram_tensor("wk", [128, KT * COLS], BF16, kind="ExternalInput")
    wv = nc.dram_tensor("wv", [128, KT * COLS], BF16, kind="ExternalInput")
    # wo rows pre-permuted h-major [h, core, 64] and pre-tiled likewise
    wo = nc.dram_tensor("wo", [128, KT * C], BF16, kind="ExternalInput")
    bqk = nc.dram_tensor("bqk", [COLS, 2], F32, kind="ExternalInput")
    bv = nc.dram_tensor("bv", [COLS, 1], F32, kind="ExternalInput")
    bo = nc.dram_tensor("bo", [1, C], BF16, kind="ExternalInput")
    mtri = nc.dram_tensor("mtri", [128, 128], BF16, kind="ExternalInput")
    emat = nc.dram_tensor("emat", [NCORES, C], BF16, kind="ExternalInput")
    # one A2A per batch; dst shard = [h0 y^T+r | pad | h1 y^T+r | pad] x 256 tok
    send = [
        nc.dram_tensor(f"a2a_send{b}", [NCORES * SH2, 256], BF16) for b in range(B)
    ]
    recv = [
        nc.dram_tensor(f"a2a_recv{b}", [NCORES * SH2, 256], BF16) for b in range(B)
    ]
    wsend = nc.dram_tensor("warm_send", [NCORES * 2, 512], BF16)
    wrecv = nc.dram_tensor("warm_recv", [NCORES * 2, 512], BF16)
    out = nc.dram_tensor("out", [ROWS_PER_CORE, C], F32, kind="ExternalOutput")

    add = mybir.AluOpType.add
    mult = mybir.AluOpType.mult

    def stage_a_batch(b, x_sb, psA, psT, cs):
        # q/k projections into transposed, head-stacked layout
        for wsb, dsb, bcol in ((cs["wq"], cs["qT"], 0), (cs["wk"], cs["kT"], 1)):
            for n in range(NCH):
                ps = psA.tile([128, 512], F32, tag="psA", name="psA", bufs=4)
                col = b * T + n * 512
                for k in range(KT):
                    nc.tensor.matmul(
                        ps[:],
                        wsb[:, k * COLS : (k + 1) * COLS],
                        x_sb[k][:, col : col + 512],
                        start=(k == 0),
                        stop=(k == KT - 1),
                    )
                nc.scalar.activation(
                    dsb[:, col : col + 512],
                    ps[:],
                    Ident,
                    bias=cs["bqk"][:, bcol : bcol + 1],
                )
        # v^T (w-stationary, wide moving), then PE-transpose to natural layout
        for n in range(NCH):
            ps = psA.tile([128, 512], F32, tag="psA", name="psA", bufs=4)
            col = b * T + n * 512
            for k in range(KT):
                nc.tensor.matmul(
                    ps[:],
                    cs["wv"][:, k * COLS : (k + 1) * COLS],
                    x_sb[k][:, col : col + 512],
                    start=(k == 0),
                    stop=(k == KT - 1),
                )
            nc.scalar.activation(
                cs["vT"][:, col : col + 512], ps[:], Ident, bias=cs["bv"][:, 0:1]
            )
        for m in range(NT):
            pv = psT.tile([128, 128], F32, tag="psT", name="psT", bufs=2)
            col = b * T + m * 128
            # out = vT_tile^T via moving identity
            nc.tensor.matmul(
                pv[:], cs["vT"][:, col : col + 128], cs["ident"][:],
                start=True, stop=True,
            )
            vi = (b * NT + m) * HL
            nc.vector.tensor_copy(
                out=cs["v"][:, vi : vi + HL, 0:D],
                in_=pv[:, :].rearrange("p (h d) -> p h d", h=HL),
            )

    def scores_chunk(b, n, pts, ptp, psS, cs):
        """Generator: score tiles + exp for query chunk (b, n); h0/h1 paired
        on disjoint PE row-groups within ONE [128,1024] PSUM tile (same-tile
        allocation keeps the pair adjacent through the scheduler, like the
        PE needs for row-group overlap). One exp covers both heads. Appends
        pt (=exp(scaled scores)^T) half-tiles per head; yields per key tile."""
        qT, kT = cs["qT"], cs["kT"]
        qcol = b * T + n * 512
        for m in range(4 * n + 4):
            kcol = b * T + m * 128
            j = m - 4 * n  # >= 0 only for the diagonal block tiles
            ps2 = psS.tile([128, 1024], F32, tag="ps2", name="ps2", bufs=3)
            for h in range(HL):
                nc.tensor.matmul(
                    ps2[:, h * 512 + max(j, 0) * 128 : (h + 1) * 512],
                    kT[h * D : (h + 1) * D, kcol : kcol + 128],
                    qT[h * D : (h + 1) * D, qcol + max(j, 0) * 128 : qcol + 512],
                    start=True,
                    stop=True,
                )
            if j >= 0:
                # causal mask of the diagonal 128-block
                for h in range(HL):
                    nc.vector.tensor_tensor(
                        ps2[:, h * 512 + j * 128 : h * 512 + (j + 1) * 128],
                        ps2[:, h * 512 + j * 128 : h * 512 + (j + 1) * 128],
                        cs["mtri"][:],
                        add,
                    )
            pt2 = ptp.tile([128, 1024], BF16, tag="pt", name="pt", bufs=28)
            if j <= 0:
                nc.scalar.activation(pt2[:], ps2[:], Exp, scale=float(SCALE))
            else:
                # skip exp over the fully-masked left columns (scalar is the
                # co-roofline engine); zero-fill them for the AV reads
                for h in range(HL):
                    nc.gpsimd.memset(pt2[:, h * 512 : h * 512 + j * 128], 0.0)
                    nc.scalar.activation(
                        pt2[:, h * 512 + j * 128 : (h + 1) * 512],
                        ps2[:, h * 512 + j * 128 : (h + 1) * 512],
                        Exp,
                        scale=float(SCALE),
                    )
            pts[0].append(pt2[:, 0:512])
            pts[1].append(pt2[:, 512:1024])
            yield

    def av_chunk(b, n, pts, psY, nrm, cs):
        """Generator: y^T (+ normalizer row) = v_aug^T @ P^T for chunk (b, n),
        then ship unnormalized y^T plus the r row (receive side divides).
        Yields after each matmul so the driver can interleave score pairs."""
        last = 4 * n + 3
        for h in range(HL):
            py = psY.tile([VW, 512], F32, tag="psY", name="psY", bufs=2)
            for m in range(4 * n + 4):
                vi = (b * NT + m) * HL + h
                nc.tensor.matmul(
                    py[:],
                    cs["v"][:, vi : vi + 1, :],
                    pts[h][m][:],
                    start=(m == 0),
                    stop=(m == last),
                )
                if m < last:
                    yield
            yn = nrm.tile([VW, 512], BF16, tag="yn", name="yn", bufs=6)
            nc.vector.tensor_copy(out=yn[:], in_=py[:])
            for p in range(2):
                dst = 2 * n + p
                o = send[b][dst * SH2 + h * SH : dst * SH2 + h * SH + VW, :]
                nc.gpsimd.dma_start(out=o, in_=yn[:, p * 256 : (p + 1) * 256])
            yield

    def a2a(b):
        nc.gpsimd.collective_compute(
            "AllToAll",
            mybir.AluOpType.bypass,
            replica_groups=[list(range(NCORES))],
            ins=[send[b][:]],
            outs=[recv[b][:]],
        )

    def make_inv(cp, r_sb, tag):
        r_f = cp.tile([NCORES, 256], F32, tag=f"rf{tag}", name=f"rf{tag}")
        nc.vector.tensor_copy(out=r_f[:], in_=r_sb[:])
        invf = cp.tile([NCORES, 256], F32, tag=f"invf{tag}", name=f"invf{tag}")
        nc.vector.reciprocal_approx_fast(out=invf[:], in_=r_f[:])
        inv = cp.tile([NCORES, 256], BF16, tag=f"inv{tag}", name=f"inv{tag}")
        nc.vector.tensor_copy(out=inv[:], in_=invf[:])
        return inv

    def stage_c_batch(b, cp, psC, cs, y_sb, yn_sb):
        """After a2a(b): unpack, normalize, Wo rows 2b,2b+1, store."""
        c0 = b * 256
        rv = recv[b].rearrange("(k p dr) c -> p dr k c", k=4, p=2)
        rr = recv[b].rearrange("(s dr) c -> s dr c", s=NCORES)
        r0_sb = cp.tile([NCORES, 256], BF16, tag=f"rsb0{b}", name=f"rsb0{b}")
        r1_sb = cp.tile([NCORES, 256], BF16, tag=f"rsb1{b}", name=f"rsb1{b}")
        # unpacks spread across the three DMA-capable queues (all idle here)
        # so the post-collective critical path is one strided DMA, not six
        nc.sync.dma_start(
            out=r0_sb[:].rearrange("s (o c) -> s o c", o=1), in_=rr[:, D : D + 1, :]
        )
        nc.scalar.dma_start(
            out=r1_sb[:].rearrange("s (o c) -> s o c", o=1),
            in_=rr[:, SH + D : SH + D + 1, :],
        )
        uq = [nc.gpsimd, nc.sync, nc.scalar, nc.gpsimd]
        for p in range(2):
            for i, (hh, k0) in enumerate(((0, 0), (SH, 4))):
                uq[2 * p + i].dma_start(
                    out=y_sb[p * D : p * D + D, k0 * 512 : (k0 + 4) * 512].rearrange(
                        "d (k c) -> d k c", k=4
                    )[:, :, c0 : c0 + 256],
                    in_=rv[p, hh : hh + D, :, :],
                )
        inv0 = make_inv(cp, r0_sb, f"0{b}")
        inv1 = make_inv(cp, r1_sb, f"1{b}")
        for k in range(8):
            inv = inv0 if k < 4 else inv1
            pb = psC.tile([128, 256], F32, tag="psB", name="psB", bufs=2)
            nc.tensor.matmul(
                pb[:],
                cs["emat"][:, (k % 4) * 128 : (k % 4 + 1) * 128],
                inv[:],
                start=True,
                stop=True,
            )
            nc.vector.tensor_tensor(
                yn_sb[:, k * 512 + c0 : k * 512 + c0 + 256],
                y_sb[:, k * 512 + c0 : k * 512 + c0 + 256],
                pb[:],
                mult,
            )
        for r in (2 * b, 2 * b + 1):
            for o in range(C // 512):
                pc = psC.tile([128, 512], F32, tag="psC", name="psC", bufs=4)
                for k in range(KT):
                    nc.tensor.matmul(
                        pc[:],
                        yn_sb[:, k * 512 + r * 128 : k * 512 + r * 128 + 128],
                        cs["wo"][:, k * C + o * 512 : k * C + (o + 1) * 512],
                        start=(k == 0),
                        stop=False,
                    )
                nc.tensor.matmul(
                    pc[:],
                    cs["ones"][0:1, :],
                    cs["bo"][0:1, o * 512 : (o + 1) * 512],
                    start=False,
                    stop=True,
                )
                osb = cp.tile([128, 512], F32, tag="osb", name="osb", bufs=3)
                nc.vector.tensor_copy(out=osb[:], in_=pc[:])
                # split the store across two queues: the final tiles are on
                # the critical tail, and one 256KB DMA is ~2x slower
                for ho, q in ((0, nc.scalar), (1, nc.gpsimd)):
                    q.dma_start(
                        out=out[
                            r * 128 : (r + 1) * 128,
                            o * 512 + ho * 256 : o * 512 + (ho + 1) * 256,
                        ],
                        in_=osb[:, ho * 256 : (ho + 1) * 256],
                    )

    with TileContext(nc) as tc:
        with tc.tile_pool(name="persist", bufs=1) as pp:
            cs = {}
            # warmup collective first: absorbs the ~11us first-trigger latency
            nc.gpsimd.collective_compute(
                "AllToAll",
                mybir.AluOpType.bypass,
                replica_groups=[list(range(NCORES))],
                ins=[wsend[:]],
                outs=[wrecv[:]],
            )
            # weights FIRST (the first q/k chains wait on these + x c0), then
            # the first x column chunks, then the small constants
            cs["wq"] = pp.tile([128, KT * COLS], BF16, tag="wq", name="wq")
            cs["wk"] = pp.tile([128, KT * COLS], BF16, tag="wk", name="wk")
            cs["wv"] = pp.tile([128, KT * COLS], BF16, tag="wv", name="wv")
            nc.sync.dma_start(out=cs["wq"][:], in_=wq[:])
            nc.scalar.dma_start(out=cs["wk"][:], in_=wk[:])
            nc.gpsimd.dma_start(out=cs["wv"][:], in_=wv[:])
            cs["ones"] = pp.tile([1, 128], BF16, tag="ones", name="ones")
            nc.vector.memset(cs["ones"][:], 1.0)
            cs["ident"] = pp.tile([128, 128], BF16, tag="ident", name="ident")
            make_identity(nc, cs["ident"][:])

            with tc.tile_pool(name="xp", bufs=1) as xp:
                # x k-tiles split into 512-col chunks, batch-0 columns first.
                # batch-1 chunks stay OFF the scalar queue: its DMA-issue
                # backpressure would head-block the q/k PSUM->SBUF copies and
                # exps that keep stage B fed.
                x_sb = [
                    xp.tile([128, B * T], BF16, tag=f"x{k}", name=f"x{k}")
                    for k in range(KT)
                ]

                def load_x(cols, queues):
                    qi = 0
                    for c in cols:
                        for k in range(KT):
                            queues[qi % len(queues)].dma_start(
                                out=x_sb[k][:, c * 512 : (c + 1) * 512],
                                in_=xT[
                                    k * 128 : (k + 1) * 128, c * 512 : (c + 1) * 512
                                ],
                            )
                            qi += 1

                cs["qT"] = pp.tile([128, B * T], BF16, tag="qT", name="qT")
                cs["kT"] = pp.tile([128, B * T], BF16, tag="kT", name="kT")
                cs["vT"] = pp.tile([128, B * T], BF16, tag="vT", name="vT")
                cs["v"] = pp.tile([128, B * NT * HL, VW], BF16, tag="v", name="v")
                nc.vector.memset(cs["v"][:], 1.0)  # presets the ones columns

                with tc.tile_pool(name="psA", bufs=1, space="PSUM") as psA, \
                     tc.tile_pool(name="psT", bufs=1, space="PSUM") as psT:
                    load_x(range(1), [nc.sync, nc.scalar, nc.gpsimd])
                    # small constants after the first-needed x chunks
                    cs["bqk"] = pp.tile([COLS, 2], F32, tag="bqk", name="bqk")
                    nc.sync.dma_start(out=cs["bqk"][:], in_=bqk[:])
                    cs["bv"] = pp.tile([COLS, 1], F32, tag="bv", name="bv")
                    nc.scalar.dma_start(out=cs["bv"][:], in_=bv[:])
                    cs["bo"] = pp.tile([1, C], BF16, tag="bo", name="bo")
                    nc.scalar.dma_start(out=cs["bo"][:], in_=bo[:])
                    cs["mtri"] = pp.tile([128, 128], BF16, tag="mtri", name="mtri")
                    nc.gpsimd.dma_start(out=cs["mtri"][:], in_=mtri[:])
                    cs["emat"] = pp.tile([NCORES, C], BF16, tag="emat", name="emat")
                    nc.gpsimd.dma_start(out=cs["emat"][:], in_=emat[:])
                    load_x(range(1, 4), [nc.sync, nc.scalar, nc.gpsimd])
                    stage_a_batch(0, x_sb, psA, psT, cs)
                    load_x(range(4, 8), [nc.sync, nc.gpsimd])
                    stage_a_batch(1, x_sb, psA, psT, cs)

                # wo issued late on scalar: behind the stage-A copies, well
                # before stage C needs it
                cs["wo"] = pp.tile([128, KT * C], BF16, tag="wo", name="wo")
                nc.scalar.dma_start(out=cs["wo"][:], in_=wo[:])

            with tc.tile_pool(name="pt", bufs=1) as ptp, tc.tile_pool(
                name="psS", bufs=1, space="PSUM"
            ) as psS, tc.tile_pool(
                name="psY", bufs=1, space="PSUM"
            ) as psY, tc.tile_pool(
                name="nrm", bufs=1
            ) as nrm:
                # software pipeline: interleave score pairs of chunk u with the
                # AV matmuls of chunk u-1 at instruction level, so the
                # scalar-engine exps (slower than paired score production)
                # overlap the AV matmul stream instead of serializing
                prev_pts = None
                for u in range(B * NCH + 1):
                    sg = None
                    if u < B * NCH:
                        cur_pts = ([], [])
                        sg = scores_chunk(u // NCH, u % NCH, cur_pts, ptp, psS, cs)
                    ag = None
                    if u > 0:
                        pb, pn = (u - 1) // NCH, (u - 1) % NCH
                        ag = av_chunk(pb, pn, prev_pts, psY, nrm, cs)
                    if sg is not None and ag is not None:
                        s_steps = 4 * (u % NCH) + 4
                        a_steps = 2 * (4 * pn + 4)
                        acc = 0.0
                        for _ in range(s_steps):
                            next(sg, None)
                            acc += a_steps / s_steps
                            while acc >= 1.0:
                                next(ag, None)
                                acc -= 1.0
                    if sg is not None:
                        for _ in sg:
                            pass
                    if ag is not None:
                        for _ in ag:
                            pass
                        if pn == NCH - 1:
                            a2a(pb)
                    prev_pts = cur_pts
            with tc.tile_pool(name="cp", bufs=1) as cp, tc.tile_pool(
                name="psC", bufs=1, space="PSUM"
            ) as psC:
                y_sb = cp.tile([128, 8 * 512], BF16, tag="ysb", name="ysb")
                yn_sb = cp.tile([128, 8 * 512], BF16, tag="ynsb", name="ynsb")
                for b in range(B):
                    stage_c_batch(b, cp, psC, cs, y_sb, yn_sb)
    nc.compile()
    return nc


def make_in_maps(x, mask, Wq, bq, Wk, bk, Wv, bv, Wo, bo):
    xT = np.ascontiguousarray(
        x.astype(np.float32).transpose(2, 0, 1).reshape(C, B * T)
    ).astype(NPBF16)
    mtri = np.where(
        np.arange(128)[:, None] > np.arange(128)[None, :], MASK_NEG, 0.0
    ).astype(NPBF16)
    # Wo rows permuted h-major: new row order = [core0 h0 d0..63, core1 h0, ...,
    # core7 h0, core0 h1, ..., core7 h1]
    perm = np.concatenate(
        [
            np.arange(c * COLS + h * D, c * COLS + h * D + D)
            for h in range(HL)
            for c in range(NCORES)
        ]
    )
    def pretile(w):
        # [C, width] -> [128, KT*width] with k-tile blocks along free axis
        width = w.shape[1]
        return np.ascontiguousarray(
            w.reshape(KT, 128, width).transpose(1, 0, 2).reshape(128, KT * width)
        )
    wo_b = pretile(Wo[perm].astype(NPBF16))
    # emat[s, c] = 1 iff shard s supplies the softmax normalizer for y-column
    # position c of the per-head k-chunk layout (chunk k = shards 2k, 2k+1)
    cols = np.arange(C)
    emat_np = (
        np.arange(NCORES)[:, None] == (2 * (cols // 128) + (cols % 128) // 64)
    ).astype(NPBF16)
    bo_b = bo.reshape(1, C).astype(NPBF16)
    in_maps = []
    for c in range(NCORES):
        cslice = slice(c * COLS, (c + 1) * COLS)
        in_maps.append(
            {
                "xT": xT,
                "wq": pretile(Wq[:, cslice].astype(NPBF16)),
                "wk": pretile(Wk[:, cslice].astype(NPBF16)),
                "wv": pretile(Wv[:, cslice].astype(NPBF16)),
                "wo": wo_b,
                "bqk": np.stack([bq[cslice], bk[cslice]], axis=1).astype(
                    np.float32
                ),
                "bv": bv[cslice].reshape(COLS, 1).astype(np.float32),
                "bo": bo_b,
                "mtri": mtri,
                "emat": emat_np,
            }
        )
    return in_maps


_CACHED_NC = None


def run(inputs, trace=False, **kw):
    global _CACHED_NC
    if _CACHED_NC is None:
        _CACHED_NC = build_nc()
    in_maps = make_in_maps(**inputs)
    res = bass_utils.run_bass_kernel_spmd(
        _CACHED_NC, in_maps, core_ids=list(range(NCORES)), trace=trace, **kw
    )
    outs = [np.asarray(res.results[c]["out"]) for c in range(NCORES)]
    full = np.empty((B, T, C), np.float32)
    for j in range(NCORES):
        full[0, 256 * j : 256 * (j + 1)] = outs[j][0:256]
        full[1, 256 * j : 256 * (j + 1)] = outs[j][256:512]
    return full, res


def kernel(**inputs):
    full, _ = run(inputs, trace=False)
    return full


# revision 20
# speedup vs baseline: 1.3572x; 1.0274x over previous
"""Causal self-attention, tensor-parallel over heads across 8 TRN2 NeuronCores.

B=2, T=2048, C=1024, H=16 heads, D=64. Each core owns 2 heads (128 cols of C)
for both batches. qT/kT hold both heads stacked on partitions (h0 rows 0:64,
h1 rows 64:128) so score matmuls pair the two heads on disjoint PE row-groups
with no duplicated bias writes. v is produced transposed (w-stationary, wide
moving) then PE-transposed to natural layout via an identity matmul. One A2A
per batch redistributes unnormalized y^T (+ softmax normalizer row) into row
shards; each core then normalizes and computes a disjoint 512-row slice of the
Wo projection. Sends ride the GpSimd queue, unpacks ride the Sync queue so
collective-completion waits never head-block compute-critical queues.
bf16 matmuls, fp32 PSUM accumulation.
"""

import sys

sys.path.insert(0, "/opt/trn_rl_repo")

import numpy as np
import ml_dtypes

import concourse.bass as bass
import concourse.bacc as bacc
import concourse.mybir as mybir
from concourse.tile import TileContext
from concourse.masks import make_identity
from concourse import bass_utils

BF16 = mybir.dt.bfloat16
F32 = mybir.dt.float32
NPBF16 = ml_dtypes.bfloat16

B, T, C, H, D = 2, 2048, 1024, 16, 64
NCORES = 8
HL = H // NCORES          # heads per core = 2
COLS = HL * D             # 128 head-cols per core
KT = C // 128             # 8 contraction k-tiles
NCH = T // 512            # 4 query chunks of 512 per batch
NT = T // 128             # 16 key tiles of 128 per batch
VW = D + 1                # 65: v columns + ones column
SH = 72                   # per-head subshard rows (65 used, pad to 4KB rows)
SH2 = 2 * SH              # both heads stacked per destination shard
ROWS_PER_CORE = B * T // NCORES  # 512 output rows per core

MASK_NEG = -60000.0
SCALE = 1.0 / np.sqrt(np.float32(D))

Exp = mybir.ActivationFunctionType.Exp
Ident = mybir.ActivationFunctionType.Identity


def build_nc():
    nc = bacc.Bacc(
        "TRN2",
        target_bir_lowering=False,
        debug=False,
        enable_asserts=False,
        num_devices=NCORES,
    )
    xT = nc.dram_tensor("xT", [C, B * T], BF16, kind="ExternalInput")
    # weights pre-tiled on host to [128, k-tile blocks] for contiguous DMA
    wq = nc.dram_tensor("wq", [128, KT * COLS], BF16, kind="ExternalInput")
    wk = nc.dram_tensor("wk", [128, KT * COLS], BF16, kind="ExternalInput")
    wv = nc.dram_tensor("wv", [128, KT * COLS], BF16, kind="ExternalInput")
    # wo rows pre-permuted h-major [h, core, 64] and pre-tiled likewise
    wo = nc.dram_tensor("wo", [128, KT * C], BF16, kind="ExternalInput")
    bqk = nc.dram_tensor("bqk", [COLS, 2], F32, kind="ExternalInput")
    bv = nc.dram_tensor("bv", [COLS, 1], F32, kind="ExternalInput")
    bo = nc.dram_tensor("bo", [1, C], BF16, kind="ExternalInput")
    mtri = nc.dram_tensor("mtri", [128, 128], BF16, kind="ExternalInput")
    emat = nc.dram_tensor("emat", [NCORES, C], BF16, kind="ExternalInput")
    # one A2A per batch; dst shard = [h0 y^T+r | pad | h1 y^T+r | pad] x 256 tok
    send = [
        nc.dram_tensor(f"a2a_send{b}", [NCORES * SH2, 256], BF16) for b in range(B)
    ]
    recv = [
        nc.dram_tensor(f"a2a_recv{b}", [NCORES * SH2, 256], BF16) for b in range(B)
    ]
    wsend = nc.dram_tensor("warm_send", [NCORES * 2, 512], BF16)
    wrecv = nc.dram_tensor("warm_recv", [NCORES * 2, 512], BF16)
    out = nc.dram_tensor("out", [ROWS_PER_CORE, C], F32, kind="ExternalOutput")

    add = mybir.AluOpType.add
    mult = mybir.AluOpType.mult

    def stage_a_batch(b, x_sb, psA, psT, cs):
        # q/k/v^T interleaved per column chunk: each landed x chunk unlocks
        # ~7us of PE work instead of ~2.4, keeping the PE ahead of the
        # x-streaming rate during warmup
        for n in range(NCH):
            col = b * T + n * 512
            for wsb, dsb, bias in (
                (cs["wq"], cs["qT"], cs["bqk"][:, 0:1]),
                (cs["wk"], cs["kT"], cs["bqk"][:, 1:2]),
                (cs["wv"], cs["vT"], cs["bv"][:, 0:1]),
            ):
                ps = psA.tile([128, 512], F32, tag="psA", name="psA", bufs=4)
                for k in range(KT):
                    nc.tensor.matmul(
                        ps[:],
                        wsb[:, k * COLS : (k + 1) * COLS],
                        x_sb[k][:, col : col + 512],
                        start=(k == 0),
                        stop=(k == KT - 1),
                    )
                # batch-1 copies land late (x-arrival paced): keep them OFF
                # the scalar queue so they can't head-block the stage-B exps
                if b == 0:
                    nc.scalar.activation(
                        dsb[:, col : col + 512], ps[:], Ident, bias=bias
                    )
                else:
                    nc.gpsimd.tensor_scalar(
                        dsb[:, col : col + 512], ps[:], bias, None, add
                    )
        for m in range(NT):
            pv = psT.tile([128, 128], F32, tag="psT", name="psT", bufs=2)
            col = b * T + m * 128
            # out = vT_tile^T via moving identity
            nc.tensor.matmul(
                pv[:], cs["vT"][:, col : col + 128], cs["ident"][:],
                start=True, stop=True,
            )
            vi = (b * NT + m) * HL
            eng = nc.vector if b == 0 else nc.gpsimd
            eng.tensor_copy(
                out=cs["v"][:, vi : vi + HL, 0:D],
                in_=pv[:, :].rearrange("p (h d) -> p h d", h=HL),
            )

    def scores_chunk(b, n, pts, ptp, psS, cs):
        """Generator: score tiles + exp for query chunk (b, n); h0/h1 paired
        on disjoint PE row-groups within ONE [128,1024] PSUM tile (same-tile
        allocation keeps the pair adjacent through the scheduler, like the
        PE needs for row-group overlap). One exp covers both heads. Appends
        pt (=exp(scaled scores)^T) half-tiles per head; yields per key tile."""
        qT, kT = cs["qT"], cs["kT"]
        qcol = b * T + n * 512
        for m in range(4 * n + 4):
            kcol = b * T + m * 128
            j = m - 4 * n  # >= 0 only for the diagonal block tiles
            ps2 = psS.tile([128, 1024], F32, tag="ps2", name="ps2", bufs=3)
            for h in range(HL):
                nc.tensor.matmul(
                    ps2[:, h * 512 + max(j, 0) * 128 : (h + 1) * 512],
                    kT[h * D : (h + 1) * D, kcol : kcol + 128],
                    qT[h * D : (h + 1) * D, qcol + max(j, 0) * 128 : qcol + 512],
                    start=True,
                    stop=True,
                )
            if j >= 0:
                # causal mask of the diagonal 128-block
                for h in range(HL):
                    nc.vector.tensor_tensor(
                        ps2[:, h * 512 + j * 128 : h * 512 + (j + 1) * 128],
                        ps2[:, h * 512 + j * 128 : h * 512 + (j + 1) * 128],
                        cs["mtri"][:],
                        add,
                    )
            pt2 = ptp.tile([128, 1024], BF16, tag="pt", name="pt", bufs=28)
            if j <= 0:
                nc.scalar.activation(pt2[:], ps2[:], Exp, scale=float(SCALE))
            else:
                # skip exp over the fully-masked left columns (scalar is the
                # co-roofline engine); zero-fill them for the AV reads
                for h in range(HL):
                    nc.vector.memset(pt2[:, h * 512 : h * 512 + j * 128], 0.0)
                    nc.scalar.activation(
                        pt2[:, h * 512 + j * 128 : (h + 1) * 512],
                        ps2[:, h * 512 + j * 128 : (h + 1) * 512],
                        Exp,
                        scale=float(SCALE),
                    )
            pts[0].append(pt2[:, 0:512])
            pts[1].append(pt2[:, 512:1024])
            yield

    def av_chunk(b, n, pts, psY, nrm, cs):
        """Generator: y^T (+ normalizer row) = v_aug^T @ P^T for chunk (b, n),
        then ship unnormalized y^T plus the r row (receive side divides).
        Yields after each matmul so the driver can interleave score pairs."""
        last = 4 * n + 3
        for h in range(HL):
            py = psY.tile([VW, 512], F32, tag="psY", name="psY", bufs=2)
            for m in range(4 * n + 4):
                vi = (b * NT + m) * HL + h
                nc.tensor.matmul(
                    py[:],
                    cs["v"][:, vi : vi + 1, 0:VW],
                    pts[h][m][:],
                    start=(m == 0),
                    stop=(m == last),
                )
                if m < last:
                    yield
            yn = nrm.tile([VW, 512], BF16, tag="yn", name="yn", bufs=6)
            nc.vector.tensor_copy(out=yn[:], in_=py[:])
            for p in range(2):
                dst = 2 * n + p
                o = send[b][dst * SH2 + h * SH : dst * SH2 + h * SH + VW, :]
                nc.gpsimd.dma_start(out=o, in_=yn[:, p * 256 : (p + 1) * 256])
            yield

    def a2a(b):
        nc.gpsimd.collective_compute(
            "AllToAll",
            mybir.AluOpType.bypass,
            replica_groups=[list(range(NCORES))],
            ins=[send[b][:]],
            outs=[recv[b][:]],
        )

    def make_inv(cp, r_sb, tag):
        r_f = cp.tile([NCORES, 256], F32, tag=f"rf{tag}", name=f"rf{tag}")
        nc.vector.tensor_copy(out=r_f[:], in_=r_sb[:])
        invf = cp.tile([NCORES, 256], F32, tag=f"invf{tag}", name=f"invf{tag}")
        nc.vector.reciprocal_approx_fast(out=invf[:], in_=r_f[:])
        inv = cp.tile([NCORES, 256], BF16, tag=f"inv{tag}", name=f"inv{tag}")
        nc.vector.tensor_copy(out=inv[:], in_=invf[:])
        return inv

    def stage_c_batch(b, cp, psC, cs, y_sb, yn_sb):
        """After a2a(b): unpack, normalize, Wo rows 2b,2b+1, store."""
        c0 = b * 256
        rv = recv[b].rearrange("(k p dr) c -> p dr k c", k=4, p=2)
        rr = recv[b].rearrange("(s dr) c -> s dr c", s=NCORES)
        r0_sb = cp.tile([NCORES, 256], BF16, tag=f"rsb0{b}", name=f"rsb0{b}")
        r1_sb = cp.tile([NCORES, 256], BF16, tag=f"rsb1{b}", name=f"rsb1{b}")
        # unpacks spread across the three DMA-capable queues (all idle here)
        # so the post-collective critical path is one strided DMA, not six
        nc.sync.dma_start(
            out=r0_sb[:].rearrange("s (o c) -> s o c", o=1), in_=rr[:, D : D + 1, :]
        )
        nc.scalar.dma_start(
            out=r1_sb[:].rearrange("s (o c) -> s o c", o=1),
            in_=rr[:, SH + D : SH + D + 1, :],
        )
        uq = [nc.gpsimd, nc.sync, nc.scalar, nc.gpsimd]
        for p in range(2):
            for i, (hh, k0) in enumerate(((0, 0), (SH, 4))):
                uq[2 * p + i].dma_start(
                    out=y_sb[p * D : p * D + D, k0 * 512 : (k0 + 4) * 512].rearrange(
                        "d (k c) -> d k c", k=4
                    )[:, :, c0 : c0 + 256],
                    in_=rv[p, hh : hh + D, :, :],
                )
        inv0 = make_inv(cp, r0_sb, f"0{b}")
        inv1 = make_inv(cp, r1_sb, f"1{b}")
        for k in range(8):
            inv = inv0 if k < 4 else inv1
            pb = psC.tile([128, 256], F32, tag="psB", name="psB", bufs=2)
            nc.tensor.matmul(
                pb[:],
                cs["emat"][:, (k % 4) * 128 : (k % 4 + 1) * 128],
                inv[:],
                start=True,
                stop=True,
            )
            nc.vector.tensor_tensor(
                yn_sb[:, k * 512 + c0 : k * 512 + c0 + 256],
                y_sb[:, k * 512 + c0 : k * 512 + c0 + 256],
                pb[:],
                mult,
            )
        for r in (2 * b, 2 * b + 1):
            for o in range(C // 512):
                pc = psC.tile([128, 512], F32, tag="psC", name="psC", bufs=4)
                for k in range(KT):
                    nc.tensor.matmul(
                        pc[:],
                        yn_sb[:, k * 512 + r * 128 : k * 512 + r * 128 + 128],
                        cs["wo"][:, k * C + o * 512 : k * C + (o + 1) * 512],
                        start=(k == 0),
                        stop=(k == KT - 1),
                    )
                osb = cp.tile([128, 512], F32, tag="osb", name="osb", bufs=3)
                nc.vector.tensor_tensor(
                    osb[:], pc[:], cs["bob"][:, o * 512 : (o + 1) * 512], add
                )
                # split the store across two queues: the final tiles are on
                # the critical tail, and one 256KB DMA is ~2x slower
                for ho, q in ((0, nc.scalar), (1, nc.gpsimd)):
                    q.dma_start(
                        out=out[
                            r * 128 : (r + 1) * 128,
                            o * 512 + ho * 256 : o * 512 + (ho + 1) * 256,
                        ],
                        in_=osb[:, ho * 256 : (ho + 1) * 256],
                    )

    with TileContext(nc) as tc:
        with tc.tile_pool(name="persist", bufs=1) as pp:
            cs = {}
            # warmup collective first: absorbs the ~11us first-trigger latency
            nc.gpsimd.collective_compute(
                "AllToAll",
                mybir.AluOpType.bypass,
                replica_groups=[list(range(NCORES))],
                ins=[wsend[:]],
                outs=[wrecv[:]],
            )
            # weights FIRST (the first q/k chains wait on these + x c0), then
            # the first x column chunks, then the small constants
            cs["wq"] = pp.tile([128, KT * COLS], BF16, tag="wq", name="wq")
            cs["wk"] = pp.tile([128, KT * COLS], BF16, tag="wk", name="wk")
            cs["wv"] = pp.tile([128, KT * COLS], BF16, tag="wv", name="wv")
            nc.sync.dma_start(out=cs["wq"][:], in_=wq[:])
            nc.scalar.dma_start(out=cs["wk"][:], in_=wk[:])
            nc.gpsimd.dma_start(out=cs["wv"][:], in_=wv[:])
            cs["ones"] = pp.tile([1, 128], BF16, tag="ones", name="ones")
            nc.vector.memset(cs["ones"][:], 1.0)
            cs["ident"] = pp.tile([128, 128], BF16, tag="ident", name="ident")
            make_identity(nc, cs["ident"][:])

            with tc.tile_pool(name="xp", bufs=1) as xp:
                # x k-tiles split into 512-col chunks, batch-0 columns first.
                # batch-1 chunks stay OFF the scalar queue: its DMA-issue
                # backpressure would head-block the q/k PSUM->SBUF copies and
                # exps that keep stage B fed.
                x_sb = [
                    xp.tile([128, B * T], BF16, tag=f"x{k}", name=f"x{k}")
                    for k in range(KT)
                ]

                def load_x(cols, queues):
                    qi = 0
                    for c in cols:
                        for k in range(KT):
                            queues[qi % len(queues)].dma_start(
                                out=x_sb[k][:, c * 512 : (c + 1) * 512],
                                in_=xT[
                                    k * 128 : (k + 1) * 128, c * 512 : (c + 1) * 512
                                ],
                            )
                            qi += 1

                cs["qT"] = pp.tile([128, B * T], BF16, tag="qT", name="qT")
                cs["kT"] = pp.tile([128, B * T], BF16, tag="kT", name="kT")
                cs["vT"] = pp.tile([128, B * T], BF16, tag="vT", name="vT")
                cs["v"] = pp.tile([128, B * NT * HL, VW + 1], BF16, tag="v", name="v")
                nc.vector.memset(cs["v"][:], 1.0)  # presets the ones columns

                with tc.tile_pool(name="psA", bufs=1, space="PSUM") as psA, \
                     tc.tile_pool(name="psT", bufs=1, space="PSUM") as psT:
                    load_x(range(1), [nc.sync, nc.scalar, nc.gpsimd])
                    # small constants after the first-needed x chunks
                    cs["bqk"] = pp.tile([COLS, 2], F32, tag="bqk", name="bqk")
                    nc.sync.dma_start(out=cs["bqk"][:], in_=bqk[:])
                    cs["bv"] = pp.tile([COLS, 1], F32, tag="bv", name="bv")
                    nc.scalar.dma_start(out=cs["bv"][:], in_=bv[:])
                    cs["bo"] = pp.tile([1, C], BF16, tag="bo", name="bo")
                    nc.scalar.dma_start(out=cs["bo"][:], in_=bo[:])
                    cs["mtri"] = pp.tile([128, 128], BF16, tag="mtri", name="mtri")
                    nc.gpsimd.dma_start(out=cs["mtri"][:], in_=mtri[:])
                    cs["emat"] = pp.tile([NCORES, C], BF16, tag="emat", name="emat")
                    nc.gpsimd.dma_start(out=cs["emat"][:], in_=emat[:])
                    load_x(range(1, 4), [nc.sync, nc.scalar, nc.gpsimd])
                    stage_a_batch(0, x_sb, psA, psT, cs)
                    load_x(range(4, 8), [nc.sync, nc.gpsimd])
                    stage_a_batch(1, x_sb, psA, psT, cs)
                    # broadcast Wo bias along partitions once, during the
                    # x-streaming window; stage C then folds it into the
                    # PSUM->SBUF move instead of a tail-critical matmul
                    cs["bob"] = pp.tile([128, C], BF16, tag="bob", name="bob")
                    for o in range(C // 512):
                        pb = psA.tile([128, 512], F32, tag="psA", name="psA", bufs=4)
                        nc.tensor.matmul(
                            pb[:],
                            cs["ones"][0:1, :],
                            cs["bo"][0:1, o * 512 : (o + 1) * 512],
                            start=True,
                            stop=True,
                        )
                        nc.vector.tensor_copy(
                            out=cs["bob"][:, o * 512 : (o + 1) * 512], in_=pb[:]
                        )

                # wo issued late on scalar: behind the stage-A copies, well
                # before stage C needs it
                cs["wo"] = pp.tile([128, KT * C], BF16, tag="wo", name="wo")
                nc.scalar.dma_start(out=cs["wo"][:], in_=wo[:])

            with tc.tile_pool(name="pt", bufs=1) as ptp, tc.tile_pool(
                name="psS", bufs=1, space="PSUM"
            ) as psS, tc.tile_pool(
                name="psY", bufs=1, space="PSUM"
            ) as psY, tc.tile_pool(
                name="nrm", bufs=1
            ) as nrm:
                # software pipeline: interleave score pairs of chunk u with the
                # AV matmuls of chunk u-1 at instruction level, so the
                # scalar-engine exps (slower than paired score production)
                # overlap the AV matmul stream instead of serializing
                prev_pts = None
                for u in range(B * NCH + 1):
                    sg = None
                    if u < B * NCH:
                        cur_pts = ([], [])
                        sg = scores_chunk(u // NCH, u % NCH, cur_pts, ptp, psS, cs)
                    ag = None
                    if u > 0:
                        pb, pn = (u - 1) // NCH, (u - 1) % NCH
                        ag = av_chunk(pb, pn, prev_pts, psY, nrm, cs)
                    if sg is not None and ag is not None:
                        s_steps = 4 * (u % NCH) + 4
                        a_steps = 2 * (4 * pn + 4)
                        acc = 0.0
                        for _ in range(s_steps):
                            next(sg, None)
                            acc += a_steps / s_steps
                            while acc >= 1.0:
                                next(ag, None)
                                acc -= 1.0
                    if sg is not None:
                        for _ in sg:
                            pass
                    if ag is not None:
                        for _ in ag:
                            pass
                        if pn == NCH - 1:
                            a2a(pb)
                    prev_pts = cur_pts
            with tc.tile_pool(name="cp", bufs=1) as cp, tc.tile_pool(
                name="psC", bufs=1, space="PSUM"
            ) as psC:
                y_sb = cp.tile([128, 8 * 512], BF16, tag="ysb", name="ysb")
                yn_sb = cp.tile([128, 8 * 512], BF16, tag="ynsb", name="ynsb")
                for b in range(B):
                    stage_c_batch(b, cp, psC, cs, y_sb, yn_sb)
    nc.compile()
    return nc


def make_in_maps(x, mask, Wq, bq, Wk, bk, Wv, bv, Wo, bo):
    xT = np.ascontiguousarray(
        x.astype(np.float32).transpose(2, 0, 1).reshape(C, B * T)
    ).astype(NPBF16)
    mtri = np.where(
        np.arange(128)[:, None] > np.arange(128)[None, :], MASK_NEG, 0.0
    ).astype(NPBF16)
    # Wo rows permuted h-major: new row order = [core0 h0 d0..63, core1 h0, ...,
    # core7 h0, core0 h1, ..., core7 h1]
    perm = np.concatenate(
        [
            np.arange(c * COLS + h * D, c * COLS + h * D + D)
            for h in range(HL)
            for c in range(NCORES)
        ]
    )
    def pretile(w):
        # [C, width] -> [128, KT*width] with k-tile blocks along free axis
        width = w.shape[1]
        return np.ascontiguousarray(
            w.reshape(KT, 128, width).transpose(1, 0, 2).reshape(128, KT * width)
        )
    wo_b = pretile(Wo[perm].astype(NPBF16))
    # emat[s, c] = 1 iff shard s supplies the softmax normalizer for y-column
    # position c of the per-head k-chunk layout (chunk k = shards 2k, 2k+1)
    cols = np.arange(C)
    emat_np = (
        np.arange(NCORES)[:, None] == (2 * (cols // 128) + (cols % 128) // 64)
    ).astype(NPBF16)
    bo_b = bo.reshape(1, C).astype(NPBF16)
    in_maps = []
    for c in range(NCORES):
        cslice = slice(c * COLS, (c + 1) * COLS)
        in_maps.append(
            {
                "xT": xT,
                "wq": pretile(Wq[:, cslice].astype(NPBF16)),
                "wk": pretile(Wk[:, cslice].astype(NPBF16)),
                "wv": pretile(Wv[:, cslice].astype(NPBF16)),
                "wo": wo_b,
                "bqk": np.stack([bq[cslice], bk[cslice]], axis=1).astype(
                    np.float32
                ),
                "bv": bv[cslice].reshape(COLS, 1).astype(np.float32),
                "bo": bo_b,
                "mtri": mtri,
                "emat": emat_np,
            }
        )
    return in_maps


_CACHED_NC = None


def run(inputs, trace=False, **kw):
    global _CACHED_NC
    if _CACHED_NC is None:
        _CACHED_NC = build_nc()
    in_maps = make_in_maps(**inputs)
    res = bass_utils.run_bass_kernel_spmd(
        _CACHED_NC, in_maps, core_ids=list(range(NCORES)), trace=trace, **kw
    )
    outs = [np.asarray(res.results[c]["out"]) for c in range(NCORES)]
    full = np.empty((B, T, C), np.float32)
    for j in range(NCORES):
        full[0, 256 * j : 256 * (j + 1)] = outs[j][0:256]
        full[1, 256 * j : 256 * (j + 1)] = outs[j][256:512]
    return full, res


def kernel(**inputs):
    full, _ = run(inputs, trace=False)
    return full
